# revision 33
# baseline (speedup 1.0000x reference)
"""Trainium2 Bass kernel for nn_AttentionSpatial (manifold attention).

Pipeline (per the reference):
  qkv = 1x1 conv -> 3x3 depthwise conv -> patchify -> per-(b,head,c) unit:
  normalize q,k -> attn = softmax(q k^T * temp) -> SPD cov -> eigh (top-100)
  -> A = U (w_fr^T w_fr) U^T -> out = A v -> re-patchify -> 1x1 conv out.

The eigendecomposition runs on the host via jax CPU float64 eigh (LAPACK
sign conventions make any on-device eigensolver unusable), and the
attention/cov pipeline upstream of it must stay f32-exact: the eigh
amplifies cov perturbations ~1e4x, so bf16/f32r/table-exp anywhere before
cov blows past the error gate.  Downstream of the eigh (phase 2) f32r is
safe.

Two device launches:
  phase1 (per-core = batch x 4-channel group): 1x1 conv (PE f32) ->
    depthwise 3x3 (DVE shifted FMAs) -> patchify gathers -> normalize ->
    exact f32 PE transposes -> attn -> softmax via direct e^{-z} poly
    (DVE/Pool split) -> centered cov.  Outputs cov + patchified v.
  phase23 (per-core = batch x 64-patch slice): Y = w_fr U^T, A^T slices,
    M = A v, channel-mixing 1x1 conv in f32r, fully contiguous output.
"""

import numpy as np

PATCH = 16
HEADS = 4
TOP_P = 100
B, C = 2, 16
NCORES = 8

_built = {}
PROFILE = False
LAST_PROFILE = []
DEBUG = {}

# direct minimax fit of e^{-z} on z in [0,2] (degree 10, f32 Horner)
NEGEXP = [0.9999999999715448, -0.9999999981035824, 0.4999999688617123,
          -0.16666644598277386, 0.041665826247535086, -0.00833141300995797,
          0.0013860949436704422, -0.00019574854948869083,
          2.3127409315526552e-05, -2.0742207592879798e-06,
          1.0360396033546386e-07]

TAPS = [(dy, dx) for dy in (-1, 0, 1) for dx in (-1, 0, 1)]


def _new_nc():
    from concourse import bacc
    return bacc.Bacc("TRN2", target_bir_lowering=False, debug=False)


# --------------------------------------------------------------------------
# phase 1: conv + patchify + attention + cov
# --------------------------------------------------------------------------

def _build_phase1():
    import concourse.bass as bass
    import concourse.tile as tile
    from concourse import mybir
    from concourse.masks import make_identity

    f32 = mybir.dt.float32
    AF = mybir.ActivationFunctionType
    OP = mybir.AluOpType
    nc = _new_nc()

    xb_d = nc.dram_tensor("xb", (16, 258, 258), f32, kind="ExternalInput")
    w1_d = nc.dram_tensor("w1big", (128, 96), f32, kind="ExternalInput")
    wdw_d = nc.dram_tensor("wdwtap", (96, 9), f32, kind="ExternalInput")
    tmp_d = nc.dram_tensor("tempu", (1, 1), f32, kind="ExternalInput")
    vpd_d = nc.dram_tensor("vpd", (4, 256, 256), f32, kind="ExternalOutput")
    cov_d = nc.dram_tensor("cov", (4, 256, 256), f32, kind="ExternalOutput")

    with tile.TileContext(nc) as tc:
        with (
            tc.tile_pool(name="big", bufs=1) as big,
            tc.tile_pool(name="unit", bufs=3) as up,
            tc.tile_pool(name="psA", bufs=2, space="PSUM") as psA,      # tr, att, conv
        ):
            ident = big.tile([128, 128], f32, tag="ident")
            make_identity(nc, ident)

            tempb = big.tile([128, 1], f32, tag="tempb")
            nc.sync.dma_start(
                tempb,
                bass.AP(tensor=tmp_d.ap().tensor, offset=0, ap=[[0, 128], [1, 1]]),
            )
            tempn = big.tile([128, 1], f32, tag="tempn")
            nc.vector.tensor_scalar_mul(tempn, tempb, -1.0)

            w1 = big.tile([128, 96], f32, tag="w1")
            nc.sync.dma_start(w1, w1_d.ap())
            wdw = big.tile([96, 9], f32, tag="wdw")
            nc.sync.dma_start(wdw, wdw_d.ap())

            # ---- x load: [part=(band8, ci16), 34 rows, 258 cols]; input is
            # host-padded to 258x258 so band bb reads padded rows 32bb..32bb+33.
            # Split into row chunks on alternating issue engines so the 1x1
            # conv can start while later rows are still in flight.
            x_sb = big.tile([128, 34, 258], f32, tag="x")
            xr = [0, 6, 12, 18, 24, 29, 34]
            xengs = [nc.sync, nc.scalar, nc.gpsimd]
            for xi in range(6):
                r0, r1 = xr[xi], xr[xi + 1]
                xengs[xi % 3].dma_start(
                    x_sb[:, r0:r1, :],
                    bass.AP(tensor=xb_d.ap().tensor, offset=r0 * 258,
                            ap=[[32 * 258, 8], [258 * 258, 16],
                                [1, (r1 - r0) * 258]]),
                )

            # ---- 1x1 conv into padded q1 [96=(o12,band8), 34, 258]
            # pad cols stay zero; pad rows come from x's zero padding.
            q1 = big.tile([96, 34, 258], f32, tag="q1")
            nc.vector.memset(q1[:, :, 0], 0.0)
            nc.vector.memset(q1[:, :, 257], 0.0)
            for ch in range(17):
                acc = psA.tile([96, 512], f32, tag="conv")
                nc.tensor.matmul(acc, w1, x_sb[:, 2 * ch:2 * ch + 2, 1:257],
                                 start=True, stop=True)
                if ch % 2 == 0:
                    nc.scalar.copy(
                        q1[:, 2 * ch:2 * ch + 2, 1:257],
                        acc.rearrange("p (a b) -> p a b", a=2))
                else:
                    nc.vector.tensor_copy(
                        q1[:, 2 * ch:2 * ch + 2, 1:257],
                        acc.rearrange("p (a b) -> p a b", a=2))

            # ---- depthwise 3x3: free-dim shifted FMA accumulation on DVE
            q2 = big.tile([96, 32, 256], f32, tag="q2")
            for t, (dy, dx) in enumerate(TAPS):
                src = q1[:, 1 + dy:33 + dy, 1 + dx:257 + dx]
                if t == 0:
                    nc.vector.tensor_scalar_mul(q2, src, wdw[:, 0:1])
                else:
                    nc.vector.scalar_tensor_tensor(
                        q2, src, wdw[:, t:t + 1], q2, op0=OP.mult, op1=OP.add)

            # ---- free-dim patchify permute (image -> patch order within each
            # band partition); after this every patch DMA is <=3 dims
            q2p = big.tile([96, 8192], f32, tag="q2p")
            for r2h in range(2):
                nc.scalar.copy(
                    q2p[:, 4096 * r2h:4096 * (r2h + 1)].rearrange(
                        "p (c b d) -> p c b d", c=16, b=16),
                    q2[:, 16 * r2h:16 * (r2h + 1), :].rearrange(
                        "p b (c d) -> p c b d", c=16))

            # ---- patchified v -> DRAM (phase2 reads it contiguously)
            with nc.allow_non_contiguous_dma("patchify scatter"):
                nc.sync.dma_start(
                    bass.AP(tensor=vpd_d.ap().tensor, offset=0,
                            ap=[[8192, 32], [1, 8192]]),
                    q2p[64:96])

                # ---- per unit: gather patches, normalize, attention, cov
                gi = 0
                for u in range(4):
                    q_pd = up.tile([128, 2, 256], f32, tag="q_pd")
                    k_pd = up.tile([128, 2, 256], f32, tag="k_pd")
                    for (osel, dst_pd) in ((u, q_pd), (4 + u, k_pd)):
                        for c2 in range(2):
                            src = q2p[osel * 8 + 4 * c2:osel * 8 + 4 * c2 + 4]
                            geng = (nc.sync, nc.scalar, nc.gpsimd)[gi % 3]
                            gi += 1
                            geng.dma_start(
                                dst_pd[:, c2, :],
                                src.rearrange("p (a b) -> p a b", a=32))

                    # row rsqrt norms (scalar sq-accum + DVE rsqrt-newton);
                    # k is scaled in place, q's scale is folded into zt below
                    scr = up.tile([128, 256], f32, tag="scr")
                    rins = {}
                    for ti, t_pd in enumerate((q_pd, k_pd)):
                        nrm = up.tile([128, 2], f32, tag=f"nrm{ti}")
                        for c2 in range(2):
                            nc.scalar.activation(scr, t_pd[:, c2, :], AF.Square,
                                                 accum_out=nrm[:, c2:c2 + 1])
                        nc.vector.tensor_scalar_max(nrm, nrm, 1e-24)
                        srt = up.tile([128, 2], f32, tag=f"srt{ti}")
                        nc.scalar.sqrt(srt, nrm)
                        rin = up.tile([128, 2], f32, tag=f"rin{ti}")
                        nc.vector.reciprocal(rin, srt)
                        nwt = up.tile([128, 2], f32, tag=f"nwt{ti}")
                        nc.vector.tensor_mul(nwt, nrm, rin)
                        nc.vector.tensor_mul(nwt, nwt, rin)
                        nc.vector.tensor_scalar(nwt, nwt, -0.5, 1.5,
                                                op0=OP.mult, op1=OP.add)
                        nc.vector.tensor_mul(rin, rin, nwt)
                        rins[ti] = rin
                    for c2 in range(2):
                        nc.scalar.activation(
                            k_pd[:, c2, :], k_pd[:, c2, :], AF.Copy,
                            scale=rins[1][:, c2:c2 + 1])
                    # tnrq[n] = -temp * rq[n] per n-chunk (for the zt fuse)
                    tnrq = up.tile([128, 2], f32, tag="tnrq")
                    nc.vector.tensor_scalar_mul(tnrq, rins[0],
                                                tempn[:, 0:1])

                    # exact f32 PE transposes -> qT,kT [d-part, dchunk, n]
                    qT = up.tile([128, 2, 256], f32, tag="qT")
                    kT = up.tile([128, 2, 256], f32, tag="kT")
                    for (src_t, dst_t) in ((q_pd, qT), (k_pd, kT)):
                        for pc in range(2):
                            for dc in range(2):
                                tp = psA.tile([128, 128], f32, tag="tr")
                                nc.tensor.transpose(
                                    tp, src_t[:, pc, 128 * dc:128 * (dc + 1)],
                                    ident)
                                nc.scalar.copy(
                                    dst_t[:, dc, 128 * pc:128 * (pc + 1)], tp)

                    # attn chunks + softmax (direct e^{-z} poly) -> xc
                    xc = up.tile([128, 2, 256], f32, tag="xc")
                    for nch in range(2):
                        att = psA.tile([128, 256], f32, tag="att")
                        for dc in range(2):
                            nc.tensor.matmul(
                                att, qT[:, dc, 128 * nch:128 * (nch + 1)],
                                kT[:, dc, :], start=(dc == 0), stop=(dc == 1))
                        # z = temp*(1 - rq[n]*att0) in [0,2] (q norm folded in)
                        zt = up.tile([128, 256], f32, tag="zt")
                        nc.vector.tensor_scalar(zt, att,
                                                tnrq[:, nch:nch + 1],
                                                tempb[:, 0:1],
                                                op0=OP.mult, op1=OP.add)
                        eng = nc.vector
                        ep = up.tile([128, 256], f32, tag="ep")
                        eng.tensor_scalar_mul(ep, zt, NEGEXP[10])
                        for kk in range(9, 0, -1):
                            eng.scalar_tensor_tensor(
                                ep, ep, NEGEXP[kk], zt, op0=OP.add, op1=OP.mult)
                        eng.tensor_scalar_add(ep, ep, NEGEXP[0])
                        # rowsum via scalar-engine accumulate
                        rssum = up.tile([128, 1], f32, tag="rssum")
                        nc.scalar.activation(scr, ep, AF.Copy,
                                             accum_out=rssum)
                        rowsum = up.tile([128, 1], f32, tag="rowsum")
                        nc.vector.reciprocal(rowsum, rssum)
                        nwt2 = up.tile([128, 1], f32, tag="nwt2")
                        nc.vector.tensor_mul(nwt2, rssum, rowsum)
                        nc.vector.tensor_scalar(nwt2, nwt2, -1.0, 2.0,
                                                op0=OP.mult, op1=OP.add)
                        nc.vector.tensor_mul(rowsum, rowsum, nwt2)
                        # xc = ep * (1/rowsum) - 1/256 (softmax rows sum to 1)
                        nc.vector.tensor_scalar(xc[:, nch, :], ep,
                                                rowsum[:, 0:1], 1.0 / 256.0,
                                                op0=OP.mult, op1=OP.subtract)

                    # xcT via exact f32 PE transposes
                    xcT = up.tile([128, 2, 256], f32, tag="xcT")
                    for pc in range(2):
                        for dc in range(2):
                            tp = psA.tile([128, 128], f32, tag="tr")
                            nc.tensor.transpose(
                                tp, xc[:, pc, 128 * dc:128 * (dc + 1)], ident)
                            nc.scalar.copy(
                                xcT[:, dc, 128 * pc:128 * (pc + 1)], tp)

                    # raw S = xc xc^T; the /trace(S) + 1e-5 I happens on the
                    # host in f64 (eigh signs are scale-invariant)
                    cov_sb = up.tile([128, 2, 256], f32, tag="cov_sb")
                    for nch in range(2):
                        cv = psA.tile([128, 256], f32, tag="att")
                        for mc in range(2):
                            nc.tensor.matmul(
                                cv, xcT[:, mc, 128 * nch:128 * (nch + 1)],
                                xcT[:, mc, :], start=(mc == 0), stop=(mc == 1))
                        if nch == 0:
                            nc.scalar.copy(cov_sb[:, nch, :], cv)
                        else:
                            nc.vector.tensor_copy(cov_sb[:, nch, :], cv)
                    nc.sync.dma_start(
                        cov_d.ap()[u].rearrange("(c p) m -> p c m", p=128),
                        cov_sb)

    nc.compile()
    return nc


# --------------------------------------------------------------------------
# phase 2+3 merged: Y = w_fr U^T, A^T slice, M slice, 1x1 conv out (f32r)
# per-core = (batch, 64-patch slice of n); host supplies full U^T of the
# batch plus the 64-column slice of it.
# --------------------------------------------------------------------------

def _build_phase23(conv_f32r=True):
    import concourse.bass as bass
    import concourse.tile as tile
    from concourse import mybir

    f32 = mybir.dt.float32
    f32r = mybir.dt.float32r
    mdt = f32r if conv_f32r else f32
    nc = _new_nc()

    ut_d = nc.dram_tensor("ut", (16, 100, 256), f32, kind="ExternalInput")
    utsl_d = nc.dram_tensor("utsl", (16, 100, 64), f32, kind="ExternalInput")
    vall_d = nc.dram_tensor("vall", (16, 256, 256), f32, kind="ExternalInput")
    wfrT_d = nc.dram_tensor("wfrT", (100, 100), f32, kind="ExternalInput")
    wpoT_d = nc.dram_tensor("wpoT", (16, 16), f32, kind="ExternalInput")
    outq_d = nc.dram_tensor("outq", (16, 64, 256), f32, kind="ExternalOutput")
    mb_d = nc.dram_tensor("mb", (16, 64, 256), f32, kind="Internal")

    with tile.TileContext(nc) as tc:
        with (
            tc.tile_pool(name="sb", bufs=1) as sb,
            tc.tile_pool(name="unit", bufs=2) as up,
            tc.tile_pool(name="ps", bufs=2, space="PSUM") as ps,    # yt, mp, op
            tc.tile_pool(name="ps1", bufs=1, space="PSUM") as ps1,  # ytn, at
        ):
            wfrT = sb.tile([100, 100], f32, tag="wfrT")
            nc.sync.dma_start(wfrT, wfrT_d.ap())
            wpoT = sb.tile([16, 16], f32, tag="wpoT")
            nc.sync.dma_start(wpoT, wpoT_d.ap())
            wpoTr = sb.tile([16, 16], mdt, tag="wpoTr")
            nc.vector.tensor_copy(wpoTr, wpoT)

            # all-units U^T loads: [100, 16u, 256] and slice [100, 16u, 64]
            ut_sb = sb.tile([100, 16, 256], f32, tag="ut_sb")
            for uh in range(4):
                (nc.sync, nc.scalar, nc.gpsimd, nc.sync)[uh].dma_start(
                    ut_sb[:, 4 * uh:4 * uh + 4, :],
                    bass.AP(tensor=ut_d.ap().tensor, offset=uh * 4 * 25600,
                            ap=[[256, 100], [25600, 4], [1, 256]]))
            utsl_sb = sb.tile([100, 16, 64], f32, tag="utsl_sb")
            nc.scalar.dma_start(
                utsl_sb,
                bass.AP(tensor=utsl_d.ap().tensor, offset=0,
                        ap=[[64, 100], [6400, 16], [1, 64]]))
            # v: [128 part(m-low), 16u, 2 mchunk, 256 d], converted to
            # f32r for the fast M matmuls
            v_sb = sb.tile([128, 16, 2, 256], f32, tag="v_sb")
            for mc in range(2):
                (nc.sync, nc.scalar)[mc].dma_start(
                    v_sb[:, :, mc, :],
                    bass.AP(tensor=vall_d.ap().tensor, offset=mc * 32768,
                            ap=[[256, 128], [65536, 16], [1, 256]]))
            vr_sb = v_sb.bitcast(mdt)
            half = 16 * 2 * 256 // 2
            vrf = vr_sb.rearrange("p a b c -> p (a b c)")
            vsf = v_sb.rearrange("p a b c -> p (a b c)")
            nc.scalar.copy(vrf[:, 0:half], vsf[:, 0:half])
            nc.vector.tensor_copy(vrf[:, half:2 * half], vsf[:, half:2 * half])

            # Y^T for all units, batched: [100, 16u, 256] (shared stationary)
            yt_all = sb.tile([100, 16, 256], f32, tag="yt_all")
            for bh in range(8):
                ytp = ps.tile([100, 512], f32, tag="yt")
                nc.tensor.matmul(ytp, wfrT,
                                 ut_sb.rearrange("p a b -> p (a b)")
                                 [:, 512 * bh:512 * (bh + 1)],
                                 start=True, stop=True)
                (nc.scalar.copy if bh % 2 == 0 else nc.vector.tensor_copy)(
                    yt_all.rearrange("p a b -> p (a b)")
                    [:, 512 * bh:512 * (bh + 1)], ytp)
            ytn_all = sb.tile([100, 16, 64], f32, tag="ytn_all")
            for bh in range(2):
                ynp = ps1.tile([100, 512], f32, tag="ytn")
                nc.tensor.matmul(ynp, wfrT,
                                 utsl_sb.rearrange("p a b -> p (a b)")
                                 [:, 512 * bh:512 * (bh + 1)],
                                 start=True, stop=True)
                (nc.scalar.copy if bh == 0 else nc.vector.tensor_copy)(
                    ytn_all.rearrange("p a b -> p (a b)")
                    [:, 512 * bh:512 * (bh + 1)], ynp)

            # mstack: [64 part(n-local), 16 c, 256 d]
            mstack = sb.tile([64, 16, 256], f32, tag="mstack")

            for u in range(16):
                # A^T slice: [m 2x128, n 64]
                at_sb = up.tile([128, 2, 64], mdt, tag="at_sb")
                for mc in range(2):
                    atp = ps1.tile([128, 64], f32, tag="at")
                    nc.tensor.matmul(atp,
                                     yt_all[:, u, 128 * mc:128 * (mc + 1)],
                                     ytn_all[:, u, :], start=True, stop=True)
                    (nc.scalar.copy if mc == 0 else nc.vector.tensor_copy)(
                        at_sb[:, mc, :], atp)

                # M slice: [n 64, d 256] = A[n,:] @ v  (lhsT = A^T chunks)
                mp = ps.tile([64, 256], f32, tag="mp")
                for mc in range(2):
                    nc.tensor.matmul(mp, at_sb[:, mc, :], vr_sb[:, u, mc, :],
                                     start=(mc == 0), stop=(mc == 1))
                (nc.scalar.copy if u % 2 == 0 else nc.vector.tensor_copy)(
                    mstack[:, u, :], mp)

            # bounce M through DRAM to move channels onto partitions
            # mb layout: [c, n_local, d]
            nc.sync.dma_start(
                bass.AP(tensor=mb_d.ap().tensor, offset=0,
                        ap=[[256, 64], [16384, 16], [1, 256]]),
                mstack)
            # conv-ready in 2 halves; the (bq,r)->(r,bq) patchify interleave
            # rides the f32r-rounding engine copy, so all DMAs are contiguous
            engs = ["scalar", "vector"]
            for half in range(2):
                m_raw = sb.tile([16, 8192], f32, tag="m_raw")
                nc.sync.dma_start(
                    m_raw,
                    bass.AP(tensor=mb_d.ap().tensor, offset=half * 8192,
                            ap=[[16384, 16], [1, 8192]]))
                m_r = sb.tile([16, 8192], mdt, tag="m_r")
                cpeng = nc.scalar if half == 0 else nc.vector
                for al in range(2):
                    dst_v = m_r[:, 4096 * al:4096 * (al + 1)].rearrange(
                        "p (b c d) -> p b c d", b=16, c=16)
                    src_v = m_raw[:, 4096 * al:4096 * (al + 1)].rearrange(
                        "p (c b d) -> p b c d", c=16, b=16)
                    if cpeng is nc.scalar:
                        cpeng.copy(dst_v, src_v)
                    else:
                        cpeng.tensor_copy(dst_v, src_v)
                o_sb = sb.tile([16, 8192], f32, tag="o_sb")
                for ch in range(16):
                    op = ps.tile([16, 512], f32, tag="op")
                    nc.tensor.matmul(op, wpoTr,
                                     m_r[:, 512 * ch:512 * (ch + 1)],
                                     start=True, stop=True)
                    eng = getattr(nc, engs[ch % 2])
                    if eng is nc.scalar:
                        eng.copy(o_sb[:, 512 * ch:512 * (ch + 1)], op)
                    else:
                        eng.tensor_copy(o_sb[:, 512 * ch:512 * (ch + 1)], op)
                nc.sync.dma_start(
                    bass.AP(tensor=outq_d.ap().tensor, offset=half * 8192,
                            ap=[[16384, 16], [1, 8192]]),
                    o_sb)

    nc.compile()
    return nc


# --------------------------------------------------------------------------
# host orchestration
# --------------------------------------------------------------------------

def _get(name):
    if name not in _built:
        if name == "p1":
            _built[name] = _build_phase1()
        else:
            _built[name] = _build_phase23(conv_f32r=True)
    return _built[name]


def make_phase1_inputs(x, w_qkv, w_dw, temperature):
    xpads = []
    for b in range(B):
        xpad = np.zeros((16, 258, 258), np.float32)
        xpad[:, 1:257, 1:257] = x[b]
        xpads.append(xpad)
    ins = []
    for k in range(NCORES):
        b, g = divmod(k, 4)
        rows = ([4 * g + u for u in range(4)]
                + [16 + 4 * g + u for u in range(4)]
                + [32 + 4 * g + u for u in range(4)])
        w1big = np.zeros((128, 96), np.float32)
        wdwtap = np.zeros((96, 9), np.float32)
        for bb in range(8):
            for o in range(12):
                w1big[bb * 16:(bb + 1) * 16, o * 8 + bb] = w_qkv[rows[o]]
                wdwtap[o * 8 + bb] = w_dw[rows[o], 0].reshape(9)
        ins.append({
            "xb": xpads[b],
            "w1big": w1big,
            "wdwtap": wdwtap,
            "tempu": np.full((1, 1), temperature[g, 0, 0], np.float32),
        })
    return ins


def _host_eigh(cov_all):
    """cov_all: raw S (32,256,256) f32 -> top-100 eigvecs of
    S/trace(S) + 1e-5 I via jax CPU f64 eigh."""
    import jax
    jax.config.update("jax_enable_x64", True)
    import jax.numpy as jnp
    cpu = jax.devices("cpu")[0]
    s = cov_all.astype(np.float64)
    tra = np.trace(s, axis1=-2, axis2=-1)[:, None, None]
    s = s / tra + 1e-5 * np.eye(256, dtype=np.float64)
    with jax.default_device(cpu):
        _, vecs = jnp.linalg.eigh(jnp.asarray(s))
        U = np.asarray(vecs)[:, :, ::-1][:, :, :TOP_P]
    return U


def _run(name, nc, in_maps):
    from concourse.bass_utils import run_bass_kernel_spmd
    r = run_bass_kernel_spmd(nc, in_maps, core_ids=list(range(NCORES)),
                             trace=PROFILE)
    if PROFILE:
        LAST_PROFILE.append((name, r))
    return r.results


def kernel(x, w_qkv, w_dw, temperature, w_fr, w_po):
    x = np.ascontiguousarray(np.asarray(x, dtype=np.float32))
    w_qkv = np.asarray(w_qkv, dtype=np.float32)
    w_dw = np.asarray(w_dw, dtype=np.float32)
    temperature = np.asarray(temperature, dtype=np.float32)
    w_fr = np.asarray(w_fr, dtype=np.float32)
    w_po = np.asarray(w_po, dtype=np.float32)

    # ---- phase 1
    nc1 = _get("p1")
    in1 = make_phase1_inputs(x, w_qkv, w_dw, temperature)
    res1 = _run("p1", nc1, in1)

    # ---- host eigh
    cov_all = np.zeros((B, C, 256, 256), np.float32)
    vfull = np.zeros((B, C, 256, 256), np.float32)
    for k in range(NCORES):
        b, g = divmod(k, 4)
        cov_all[b, 4 * g:4 * g + 4] = res1[k]["cov"]
        vfull[b, 4 * g:4 * g + 4] = res1[k]["vpd"]
    DEBUG["cov_all"] = cov_all
    DEBUG["vfull"] = vfull
    U = _host_eigh(cov_all.reshape(-1, 256, 256))
    UT = np.ascontiguousarray(
        U.transpose(0, 2, 1).astype(np.float32)).reshape(B, C, TOP_P, 256)

    # ---- phase 2+3
    nc23 = _get("p23")
    wfrT = np.ascontiguousarray(w_fr.T)
    wpoT = np.ascontiguousarray(w_po.T)
    in23 = []
    for k in range(NCORES):
        b, qr = divmod(k, 4)
        in23.append({
            "ut": UT[b],
            "utsl": np.ascontiguousarray(UT[b][:, :, 64 * qr:64 * (qr + 1)]),
            "vall": vfull[b],
            "wfrT": wfrT,
            "wpoT": wpoT,
        })
    try:
        res23 = _run("p23", nc23, in23)
    except Exception:
        # f32r DMA provenance may be rejected by the BIR verifier at
        # NEFF-compile time; fall back to a plain-f32 conv build.
        _built["p23"] = _build_phase23(conv_f32r=False)
        res23 = _run("p23", _built["p23"], in23)

    out = np.zeros((B, C, 256, 256), np.float32)
    for k in range(NCORES):
        b, qr = divmod(k, 4)
        out[b, :, 64 * qr:64 * (qr + 1), :] = res23[k]["outq"]
    return out


# revision 34
# speedup vs baseline: 1.1566x; 1.1566x over previous
"""Trainium2 Bass kernel for nn_AttentionSpatial (manifold attention).

Pipeline (per the reference):
  qkv = 1x1 conv -> 3x3 depthwise conv -> patchify -> per-(b,head,c) unit:
  normalize q,k -> attn = softmax(q k^T * temp) -> SPD cov -> eigh (top-100)
  -> A = U (w_fr^T w_fr) U^T -> out = A v -> re-patchify -> 1x1 conv out.

The eigendecomposition runs on the host via jax CPU float64 eigh (LAPACK
sign conventions make any on-device eigensolver unusable), and the
attention/cov pipeline upstream of it must stay f32-exact: the eigh
amplifies cov perturbations ~1e4x, so bf16/f32r/table-exp anywhere before
cov blows past the error gate.  Downstream of the eigh (phase 2) f32r is
safe.

Two device launches:
  phase1 (per-core = batch x 4-channel group): 1x1 conv (PE f32) ->
    depthwise 3x3 (DVE shifted FMAs) -> patchify gathers -> normalize ->
    exact f32 PE transposes -> attn -> softmax via direct e^{-z} poly
    (DVE/Pool split) -> centered cov.  Outputs cov + patchified v.
  phase23 (per-core = batch x 64-patch slice): Y = w_fr U^T, A^T slices,
    M = A v, channel-mixing 1x1 conv in f32r, fully contiguous output.
"""

import numpy as np

PATCH = 16
HEADS = 4
TOP_P = 100
B, C = 2, 16
NCORES = 8

_built = {}
PROFILE = False
LAST_PROFILE = []
DEBUG = {}

# direct minimax fit of e^{-z} on z in [0,2] (degree 10, f32 Horner)
NEGEXP = [0.9999999999715448, -0.9999999981035824, 0.4999999688617123,
          -0.16666644598277386, 0.041665826247535086, -0.00833141300995797,
          0.0013860949436704422, -0.00019574854948869083,
          2.3127409315526552e-05, -2.0742207592879798e-06,
          1.0360396033546386e-07]

TAPS = [(dy, dx) for dy in (-1, 0, 1) for dx in (-1, 0, 1)]


def _new_nc():
    from concourse import bacc
    return bacc.Bacc("TRN2", target_bir_lowering=False, debug=False)


# --------------------------------------------------------------------------
# phase 1: conv + patchify + attention + cov
# --------------------------------------------------------------------------

def _build_phase1():
    import concourse.bass as bass
    import concourse.tile as tile
    from concourse import mybir
    from concourse.masks import make_identity

    f32 = mybir.dt.float32
    AF = mybir.ActivationFunctionType
    OP = mybir.AluOpType
    nc = _new_nc()

    xb_d = nc.dram_tensor("xb", (16, 258, 258), f32, kind="ExternalInput")
    w1_d = nc.dram_tensor("w1big", (128, 96), f32, kind="ExternalInput")
    wdw_d = nc.dram_tensor("wdwtap", (96, 9), f32, kind="ExternalInput")
    tmp_d = nc.dram_tensor("tempu", (1, 1), f32, kind="ExternalInput")
    vpd_d = nc.dram_tensor("vpd", (4, 256, 256), f32, kind="ExternalOutput")
    cov_d = nc.dram_tensor("cov", (4, 256, 256), f32, kind="ExternalOutput")

    with tile.TileContext(nc) as tc:
        with (
            tc.tile_pool(name="big", bufs=1) as big,
            tc.tile_pool(name="unit", bufs=3) as up,
            tc.tile_pool(name="psA", bufs=2, space="PSUM") as psA,      # tr, att, conv
        ):
            ident = big.tile([128, 128], f32, tag="ident")
            make_identity(nc, ident)

            tempb = big.tile([128, 1], f32, tag="tempb")
            nc.sync.dma_start(
                tempb,
                bass.AP(tensor=tmp_d.ap().tensor, offset=0, ap=[[0, 128], [1, 1]]),
            )
            tempn = big.tile([128, 1], f32, tag="tempn")
            nc.vector.tensor_scalar_mul(tempn, tempb, -1.0)

            w1 = big.tile([128, 96], f32, tag="w1")
            nc.sync.dma_start(w1, w1_d.ap())
            wdw = big.tile([96, 9], f32, tag="wdw")
            nc.sync.dma_start(wdw, wdw_d.ap())

            # ---- x load: [part=(band8, ci16), 34 rows, 258 cols]; input is
            # host-padded to 258x258 so band bb reads padded rows 32bb..32bb+33.
            # Split into row chunks on alternating issue engines so the 1x1
            # conv can start while later rows are still in flight.
            x_sb = big.tile([128, 34, 258], f32, tag="x")
            xr = [0, 6, 12, 18, 24, 29, 34]
            xengs = [nc.sync, nc.scalar, nc.gpsimd]
            for xi in range(6):
                r0, r1 = xr[xi], xr[xi + 1]
                xengs[xi % 3].dma_start(
                    x_sb[:, r0:r1, :],
                    bass.AP(tensor=xb_d.ap().tensor, offset=r0 * 258,
                            ap=[[32 * 258, 8], [258 * 258, 16],
                                [1, (r1 - r0) * 258]]),
                )

            # ---- 1x1 conv into padded q1 [96=(o12,band8), 34, 258]
            # pad cols stay zero; pad rows come from x's zero padding.
            q1 = big.tile([96, 34, 258], f32, tag="q1")
            nc.vector.memset(q1[:, :, 0], 0.0)
            nc.vector.memset(q1[:, :, 257], 0.0)
            for ch in range(17):
                acc = psA.tile([96, 512], f32, tag="conv")
                nc.tensor.matmul(acc, w1, x_sb[:, 2 * ch:2 * ch + 2, 1:257],
                                 start=True, stop=True)
                if ch % 2 == 0:
                    nc.scalar.copy(
                        q1[:, 2 * ch:2 * ch + 2, 1:257],
                        acc.rearrange("p (a b) -> p a b", a=2))
                else:
                    nc.vector.tensor_copy(
                        q1[:, 2 * ch:2 * ch + 2, 1:257],
                        acc.rearrange("p (a b) -> p a b", a=2))

            # ---- depthwise 3x3: free-dim shifted FMA accumulation on DVE
            q2 = big.tile([96, 32, 256], f32, tag="q2")
            for t, (dy, dx) in enumerate(TAPS):
                src = q1[:, 1 + dy:33 + dy, 1 + dx:257 + dx]
                if t == 0:
                    nc.vector.tensor_scalar_mul(q2, src, wdw[:, 0:1])
                else:
                    nc.vector.scalar_tensor_tensor(
                        q2, src, wdw[:, t:t + 1], q2, op0=OP.mult, op1=OP.add)

            # ---- free-dim patchify permute (image -> patch order within each
            # band partition); after this every patch DMA is <=3 dims
            q2p = big.tile([96, 8192], f32, tag="q2p")
            for r2h in range(2):
                nc.scalar.copy(
                    q2p[:, 4096 * r2h:4096 * (r2h + 1)].rearrange(
                        "p (c b d) -> p c b d", c=16, b=16),
                    q2[:, 16 * r2h:16 * (r2h + 1), :].rearrange(
                        "p b (c d) -> p c b d", c=16))

            # ---- patchified v -> DRAM (phase2 reads it contiguously)
            with nc.allow_non_contiguous_dma("patchify scatter"):
                nc.sync.dma_start(
                    bass.AP(tensor=vpd_d.ap().tensor, offset=0,
                            ap=[[8192, 32], [1, 8192]]),
                    q2p[64:96])

                # ---- per unit: gather patches, normalize, attention, cov
                gi = 0
                for u in range(4):
                    q_pd = up.tile([128, 2, 256], f32, tag="q_pd")
                    k_pd = up.tile([128, 2, 256], f32, tag="k_pd")
                    for (osel, dst_pd) in ((u, q_pd), (4 + u, k_pd)):
                        for c2 in range(2):
                            src = q2p[osel * 8 + 4 * c2:osel * 8 + 4 * c2 + 4]
                            geng = (nc.sync, nc.scalar, nc.gpsimd)[gi % 3]
                            gi += 1
                            geng.dma_start(
                                dst_pd[:, c2, :],
                                src.rearrange("p (a b) -> p a b", a=32))

                    # row rsqrt norms (scalar sq-accum + DVE rsqrt-newton);
                    # k is scaled in place, q's scale is folded into zt below
                    scr = up.tile([128, 256], f32, tag="scr")
                    rins = {}
                    for ti, t_pd in enumerate((q_pd, k_pd)):
                        nrm = up.tile([128, 2], f32, tag=f"nrm{ti}")
                        for c2 in range(2):
                            nc.scalar.activation(scr, t_pd[:, c2, :], AF.Square,
                                                 accum_out=nrm[:, c2:c2 + 1])
                        nc.vector.tensor_scalar_max(nrm, nrm, 1e-24)
                        srt = up.tile([128, 2], f32, tag=f"srt{ti}")
                        nc.scalar.sqrt(srt, nrm)
                        rin = up.tile([128, 2], f32, tag=f"rin{ti}")
                        nc.vector.reciprocal(rin, srt)
                        nwt = up.tile([128, 2], f32, tag=f"nwt{ti}")
                        nc.vector.tensor_mul(nwt, nrm, rin)
                        nc.vector.tensor_mul(nwt, nwt, rin)
                        nc.vector.tensor_scalar(nwt, nwt, -0.5, 1.5,
                                                op0=OP.mult, op1=OP.add)
                        nc.vector.tensor_mul(rin, rin, nwt)
                        rins[ti] = rin
                    for c2 in range(2):
                        nc.scalar.activation(
                            k_pd[:, c2, :], k_pd[:, c2, :], AF.Copy,
                            scale=rins[1][:, c2:c2 + 1])
                    # tnrq[n] = -temp * rq[n] per n-chunk (for the zt fuse)
                    tnrq = up.tile([128, 2], f32, tag="tnrq")
                    nc.vector.tensor_scalar_mul(tnrq, rins[0],
                                                tempn[:, 0:1])

                    # exact f32 PE transposes -> qT,kT [d-part, dchunk, n]
                    qT = up.tile([128, 2, 256], f32, tag="qT")
                    kT = up.tile([128, 2, 256], f32, tag="kT")
                    for (src_t, dst_t) in ((q_pd, qT), (k_pd, kT)):
                        for pc in range(2):
                            for dc in range(2):
                                tp = psA.tile([128, 128], f32, tag="tr")
                                nc.tensor.transpose(
                                    tp, src_t[:, pc, 128 * dc:128 * (dc + 1)],
                                    ident)
                                nc.scalar.copy(
                                    dst_t[:, dc, 128 * pc:128 * (pc + 1)], tp)

                    # attn chunks + softmax (direct e^{-z} poly) -> xc
                    xc = up.tile([128, 2, 256], f32, tag="xc")
                    for nch in range(2):
                        att = psA.tile([128, 256], f32, tag="att")
                        for dc in range(2):
                            nc.tensor.matmul(
                                att, qT[:, dc, 128 * nch:128 * (nch + 1)],
                                kT[:, dc, :], start=(dc == 0), stop=(dc == 1))
                        # z = temp*(1 - rq[n]*att0) in [0,2] (q norm folded in)
                        zt = up.tile([128, 256], f32, tag="zt")
                        nc.vector.tensor_scalar(zt, att,
                                                tnrq[:, nch:nch + 1],
                                                tempb[:, 0:1],
                                                op0=OP.mult, op1=OP.add)
                        eng = nc.vector
                        ep = up.tile([128, 256], f32, tag="ep")
                        eng.tensor_scalar_mul(ep, zt, NEGEXP[10])
                        for kk in range(9, 0, -1):
                            eng.scalar_tensor_tensor(
                                ep, ep, NEGEXP[kk], zt, op0=OP.add, op1=OP.mult)
                        eng.tensor_scalar_add(ep, ep, NEGEXP[0])
                        # rowsum via scalar-engine accumulate
                        rssum = up.tile([128, 1], f32, tag="rssum")
                        nc.scalar.activation(scr, ep, AF.Copy,
                                             accum_out=rssum)
                        rowsum = up.tile([128, 1], f32, tag="rowsum")
                        nc.vector.reciprocal(rowsum, rssum)
                        nwt2 = up.tile([128, 1], f32, tag="nwt2")
                        nc.vector.tensor_mul(nwt2, rssum, rowsum)
                        nc.vector.tensor_scalar(nwt2, nwt2, -1.0, 2.0,
                                                op0=OP.mult, op1=OP.add)
                        nc.vector.tensor_mul(rowsum, rowsum, nwt2)
                        # xc = ep * (1/rowsum) - 1/256 (softmax rows sum to 1)
                        nc.vector.tensor_scalar(xc[:, nch, :], ep,
                                                rowsum[:, 0:1], 1.0 / 256.0,
                                                op0=OP.mult, op1=OP.subtract)

                    # xcT via exact f32 PE transposes
                    xcT = up.tile([128, 2, 256], f32, tag="xcT")
                    for pc in range(2):
                        for dc in range(2):
                            tp = psA.tile([128, 128], f32, tag="tr")
                            nc.tensor.transpose(
                                tp, xc[:, pc, 128 * dc:128 * (dc + 1)], ident)
                            nc.scalar.copy(
                                xcT[:, dc, 128 * pc:128 * (pc + 1)], tp)

                    # raw S = xc xc^T; the /trace(S) + 1e-5 I happens on the
                    # host in f64 (eigh signs are scale-invariant)
                    cov_sb = up.tile([128, 2, 256], f32, tag="cov_sb")
                    for nch in range(2):
                        cv = psA.tile([128, 256], f32, tag="att")
                        for mc in range(2):
                            nc.tensor.matmul(
                                cv, xcT[:, mc, 128 * nch:128 * (nch + 1)],
                                xcT[:, mc, :], start=(mc == 0), stop=(mc == 1))
                        if nch == 0:
                            nc.scalar.copy(cov_sb[:, nch, :], cv)
                        else:
                            nc.vector.tensor_copy(cov_sb[:, nch, :], cv)
                    nc.sync.dma_start(
                        cov_d.ap()[u].rearrange("(c p) m -> p c m", p=128),
                        cov_sb)

    nc.compile()
    return nc


# --------------------------------------------------------------------------
# phase 2+3 merged: Y = w_fr U^T, A^T slice, M slice, 1x1 conv out (f32r)
# per-core = (batch, 64-patch slice of n); host supplies full U^T of the
# batch plus the 64-column slice of it.
# --------------------------------------------------------------------------

def _build_phase23(conv_f32r=True):
    import concourse.bass as bass
    import concourse.tile as tile
    from concourse import mybir

    f32 = mybir.dt.float32
    f32r = mybir.dt.float32r
    mdt = f32r if conv_f32r else f32
    nc = _new_nc()

    ut_d = nc.dram_tensor("ut", (16, 100, 256), f32, kind="ExternalInput")
    utsl_d = nc.dram_tensor("utsl", (16, 100, 64), f32, kind="ExternalInput")
    vall_d = nc.dram_tensor("vall", (16, 256, 256), f32, kind="ExternalInput")
    wfrT_d = nc.dram_tensor("wfrT", (100, 100), f32, kind="ExternalInput")
    wpoT_d = nc.dram_tensor("wpoT", (16, 16), f32, kind="ExternalInput")
    outq_d = nc.dram_tensor("outq", (16, 64, 256), f32, kind="ExternalOutput")
    mb_d = nc.dram_tensor("mb", (16, 64, 256), f32, kind="Internal")

    with tile.TileContext(nc) as tc:
        with (
            tc.tile_pool(name="sb", bufs=1) as sb,
            tc.tile_pool(name="unit", bufs=2) as up,
            tc.tile_pool(name="ps", bufs=2, space="PSUM") as ps,    # yt, mp, op
            tc.tile_pool(name="ps1", bufs=1, space="PSUM") as ps1,  # ytn, at
        ):
            wfrT = sb.tile([100, 100], f32, tag="wfrT")
            nc.sync.dma_start(wfrT, wfrT_d.ap())
            wpoT = sb.tile([16, 16], f32, tag="wpoT")
            nc.sync.dma_start(wpoT, wpoT_d.ap())
            wpoTr = sb.tile([16, 16], mdt, tag="wpoTr")
            nc.vector.tensor_copy(wpoTr, wpoT)

            # all-units U^T loads: [100, 16u, 256] and slice [100, 16u, 64]
            ut_sb = sb.tile([100, 16, 256], f32, tag="ut_sb")
            for uh in range(4):
                (nc.sync, nc.scalar, nc.gpsimd, nc.sync)[uh].dma_start(
                    ut_sb[:, 4 * uh:4 * uh + 4, :],
                    bass.AP(tensor=ut_d.ap().tensor, offset=uh * 4 * 25600,
                            ap=[[256, 100], [25600, 4], [1, 256]]))
            utsl_sb = sb.tile([100, 16, 64], f32, tag="utsl_sb")
            nc.scalar.dma_start(
                utsl_sb,
                bass.AP(tensor=utsl_d.ap().tensor, offset=0,
                        ap=[[64, 100], [6400, 16], [1, 64]]))

            # Y^T for all units, batched: [100, 16u, 256] (shared stationary)
            yt_all = sb.tile([100, 16, 256], f32, tag="yt_all")
            for bh in range(8):
                ytp = ps.tile([100, 512], f32, tag="yt")
                nc.tensor.matmul(ytp, wfrT,
                                 ut_sb.rearrange("p a b -> p (a b)")
                                 [:, 512 * bh:512 * (bh + 1)],
                                 start=True, stop=True)
                (nc.scalar.copy if bh % 2 == 0 else nc.vector.tensor_copy)(
                    yt_all.rearrange("p a b -> p (a b)")
                    [:, 512 * bh:512 * (bh + 1)], ytp)
            ytn_all = sb.tile([100, 16, 64], f32, tag="ytn_all")
            for bh in range(2):
                ynp = ps1.tile([100, 512], f32, tag="ytn")
                nc.tensor.matmul(ynp, wfrT,
                                 utsl_sb.rearrange("p a b -> p (a b)")
                                 [:, 512 * bh:512 * (bh + 1)],
                                 start=True, stop=True)
                (nc.scalar.copy if bh == 0 else nc.vector.tensor_copy)(
                    ytn_all.rearrange("p a b -> p (a b)")
                    [:, 512 * bh:512 * (bh + 1)], ynp)

            # mstack: [64 part(n-local), 16 c, 256 d]
            mstack = sb.tile([64, 16, 256], f32, tag="mstack")

            for u in range(16):
                # per-unit v load + f32r rounding
                v_u = up.tile([128, 2, 256], f32, tag="v_u")
                (nc.sync, nc.scalar, nc.gpsimd)[u % 3].dma_start(
                    v_u,
                    bass.AP(tensor=vall_d.ap().tensor, offset=u * 65536,
                            ap=[[256, 128], [32768, 2], [1, 256]]))
                vr_u = up.tile([128, 2, 256], mdt, tag="vr_u")
                if u % 2 == 0:
                    nc.scalar.copy(
                        vr_u.rearrange("p a b -> p (a b)"),
                        v_u.rearrange("p a b -> p (a b)"))
                else:
                    nc.vector.tensor_copy(
                        vr_u.rearrange("p a b -> p (a b)"),
                        v_u.rearrange("p a b -> p (a b)"))

                # A^T slice: [m 2x128, n 64]
                at_sb = up.tile([128, 2, 64], mdt, tag="at_sb")
                for mc in range(2):
                    atp = ps1.tile([128, 64], f32, tag="at")
                    nc.tensor.matmul(atp,
                                     yt_all[:, u, 128 * mc:128 * (mc + 1)],
                                     ytn_all[:, u, :], start=True, stop=True)
                    (nc.scalar.copy if mc == 0 else nc.vector.tensor_copy)(
                        at_sb[:, mc, :], atp)

                # M slice: [n 64, d 256] = A[n,:] @ v  (lhsT = A^T chunks)
                mp = ps.tile([64, 256], f32, tag="mp")
                for mc in range(2):
                    nc.tensor.matmul(mp, at_sb[:, mc, :], vr_u[:, mc, :],
                                     start=(mc == 0), stop=(mc == 1))
                (nc.scalar.copy if u % 2 == 0 else nc.vector.tensor_copy)(
                    mstack[:, u, :], mp)

            # bounce M through DRAM to move channels onto partitions
            # mb layout: [c, n_local, d]
            nc.sync.dma_start(
                bass.AP(tensor=mb_d.ap().tensor, offset=0,
                        ap=[[256, 64], [16384, 16], [1, 256]]),
                mstack)
            # conv-ready in 2 halves; the (bq,r)->(r,bq) patchify interleave
            # rides the f32r-rounding engine copy, so all DMAs are contiguous
            engs = ["scalar", "vector"]
            for half in range(2):
                m_raw = sb.tile([16, 8192], f32, tag="m_raw")
                nc.sync.dma_start(
                    m_raw,
                    bass.AP(tensor=mb_d.ap().tensor, offset=half * 8192,
                            ap=[[16384, 16], [1, 8192]]))
                m_r = sb.tile([16, 8192], mdt, tag="m_r")
                cpeng = nc.scalar if half == 0 else nc.vector
                for al in range(2):
                    dst_v = m_r[:, 4096 * al:4096 * (al + 1)].rearrange(
                        "p (b c d) -> p b c d", b=16, c=16)
                    src_v = m_raw[:, 4096 * al:4096 * (al + 1)].rearrange(
                        "p (c b d) -> p b c d", c=16, b=16)
                    if cpeng is nc.scalar:
                        cpeng.copy(dst_v, src_v)
                    else:
                        cpeng.tensor_copy(dst_v, src_v)
                o_sb = sb.tile([16, 8192], f32, tag="o_sb")
                for ch in range(16):
                    op = ps.tile([16, 512], f32, tag="op")
                    nc.tensor.matmul(op, wpoTr,
                                     m_r[:, 512 * ch:512 * (ch + 1)],
                                     start=True, stop=True)
                    eng = getattr(nc, engs[ch % 2])
                    if eng is nc.scalar:
                        eng.copy(o_sb[:, 512 * ch:512 * (ch + 1)], op)
                    else:
                        eng.tensor_copy(o_sb[:, 512 * ch:512 * (ch + 1)], op)
                nc.sync.dma_start(
                    bass.AP(tensor=outq_d.ap().tensor, offset=half * 8192,
                            ap=[[16384, 16], [1, 8192]]),
                    o_sb)

    nc.compile()
    return nc


# --------------------------------------------------------------------------
# host orchestration
# --------------------------------------------------------------------------

def _get(name):
    if name not in _built:
        if name == "p1":
            _built[name] = _build_phase1()
        else:
            _built[name] = _build_phase23(conv_f32r=True)
    return _built[name]


def make_phase1_inputs(x, w_qkv, w_dw, temperature):
    xpads = []
    for b in range(B):
        xpad = np.zeros((16, 258, 258), np.float32)
        xpad[:, 1:257, 1:257] = x[b]
        xpads.append(xpad)
    ins = []
    for k in range(NCORES):
        b, g = divmod(k, 4)
        rows = ([4 * g + u for u in range(4)]
                + [16 + 4 * g + u for u in range(4)]
                + [32 + 4 * g + u for u in range(4)])
        w1big = np.zeros((128, 96), np.float32)
        wdwtap = np.zeros((96, 9), np.float32)
        for bb in range(8):
            for o in range(12):
                w1big[bb * 16:(bb + 1) * 16, o * 8 + bb] = w_qkv[rows[o]]
                wdwtap[o * 8 + bb] = w_dw[rows[o], 0].reshape(9)
        ins.append({
            "xb": xpads[b],
            "w1big": w1big,
            "wdwtap": wdwtap,
            "tempu": np.full((1, 1), temperature[g, 0, 0], np.float32),
        })
    return ins


def _host_eigh(cov_all):
    """cov_all: raw S (32,256,256) f32 -> top-100 eigvecs of
    S/trace(S) + 1e-5 I via jax CPU f64 eigh."""
    import jax
    jax.config.update("jax_enable_x64", True)
    import jax.numpy as jnp
    cpu = jax.devices("cpu")[0]
    s = cov_all.astype(np.float64)
    tra = np.trace(s, axis1=-2, axis2=-1)[:, None, None]
    s = s / tra + 1e-5 * np.eye(256, dtype=np.float64)
    with jax.default_device(cpu):
        _, vecs = jnp.linalg.eigh(jnp.asarray(s))
        U = np.asarray(vecs)[:, :, ::-1][:, :, :TOP_P]
    return U


def _run(name, nc, in_maps):
    from concourse.bass_utils import run_bass_kernel_spmd
    r = run_bass_kernel_spmd(nc, in_maps, core_ids=list(range(NCORES)),
                             trace=PROFILE)
    if PROFILE:
        LAST_PROFILE.append((name, r))
    return r.results


def kernel(x, w_qkv, w_dw, temperature, w_fr, w_po):
    x = np.ascontiguousarray(np.asarray(x, dtype=np.float32))
    w_qkv = np.asarray(w_qkv, dtype=np.float32)
    w_dw = np.asarray(w_dw, dtype=np.float32)
    temperature = np.asarray(temperature, dtype=np.float32)
    w_fr = np.asarray(w_fr, dtype=np.float32)
    w_po = np.asarray(w_po, dtype=np.float32)

    # ---- phase 1
    nc1 = _get("p1")
    in1 = make_phase1_inputs(x, w_qkv, w_dw, temperature)
    res1 = _run("p1", nc1, in1)

    # ---- host eigh
    cov_all = np.zeros((B, C, 256, 256), np.float32)
    vfull = np.zeros((B, C, 256, 256), np.float32)
    for k in range(NCORES):
        b, g = divmod(k, 4)
        cov_all[b, 4 * g:4 * g + 4] = res1[k]["cov"]
        vfull[b, 4 * g:4 * g + 4] = res1[k]["vpd"]
    DEBUG["cov_all"] = cov_all
    DEBUG["vfull"] = vfull
    U = _host_eigh(cov_all.reshape(-1, 256, 256))
    UT = np.ascontiguousarray(
        U.transpose(0, 2, 1).astype(np.float32)).reshape(B, C, TOP_P, 256)

    # ---- phase 2+3
    nc23 = _get("p23")
    wfrT = np.ascontiguousarray(w_fr.T)
    wpoT = np.ascontiguousarray(w_po.T)
    in23 = []
    for k in range(NCORES):
        b, qr = divmod(k, 4)
        in23.append({
            "ut": UT[b],
            "utsl": np.ascontiguousarray(UT[b][:, :, 64 * qr:64 * (qr + 1)]),
            "vall": vfull[b],
            "wfrT": wfrT,
            "wpoT": wpoT,
        })
    try:
        res23 = _run("p23", nc23, in23)
    except Exception:
        # f32r DMA provenance may be rejected by the BIR verifier at
        # NEFF-compile time; fall back to a plain-f32 conv build.
        _built["p23"] = _build_phase23(conv_f32r=False)
        res23 = _run("p23", _built["p23"], in23)

    out = np.zeros((B, C, 256, 256), np.float32)
    for k in range(NCORES):
        b, qr = divmod(k, 4)
        out[b, :, 64 * qr:64 * (qr + 1), :] = res23[k]["outq"]
    return out


# revision 37
# speedup vs baseline: 1.1871x; 1.0264x over previous
"""Trainium2 Bass kernel for nn_AttentionSpatial (manifold attention).

Pipeline (per the reference):
  qkv = 1x1 conv -> 3x3 depthwise conv -> patchify -> per-(b,head,c) unit:
  normalize q,k -> attn = softmax(q k^T * temp) -> SPD cov -> eigh (top-100)
  -> A = U (w_fr^T w_fr) U^T -> out = A v -> re-patchify -> 1x1 conv out.

The eigendecomposition runs on the host via jax CPU float64 eigh (LAPACK
sign conventions make any on-device eigensolver unusable), and the
attention/cov pipeline upstream of it must stay f32-exact: the eigh
amplifies cov perturbations ~1e4x, so bf16/f32r/table-exp anywhere before
cov blows past the error gate.  Downstream of the eigh (phase 2) f32r is
safe.

Two device launches:
  phase1 (per-core = batch x 4-channel group): 1x1 conv (PE f32) ->
    depthwise 3x3 (DVE shifted FMAs) -> patchify gathers -> normalize ->
    exact f32 PE transposes -> attn -> softmax via direct e^{-z} poly
    (DVE/Pool split) -> centered cov.  Outputs cov + patchified v.
  phase23 (per-core = batch x 64-patch slice): Y = w_fr U^T, A^T slices,
    M = A v, channel-mixing 1x1 conv in f32r, fully contiguous output.
"""

import numpy as np

PATCH = 16
HEADS = 4
TOP_P = 100
B, C = 2, 16
NCORES = 8

_built = {}
PROFILE = False
LAST_PROFILE = []
DEBUG = {}

# direct minimax fit of e^{-z} on z in [0,2] (degree 10, f32 Horner)
NEGEXP = [0.9999999999715448, -0.9999999981035824, 0.4999999688617123,
          -0.16666644598277386, 0.041665826247535086, -0.00833141300995797,
          0.0013860949436704422, -0.00019574854948869083,
          2.3127409315526552e-05, -2.0742207592879798e-06,
          1.0360396033546386e-07]

TAPS = [(dy, dx) for dy in (-1, 0, 1) for dx in (-1, 0, 1)]


def _new_nc():
    from concourse import bacc
    return bacc.Bacc("TRN2", target_bir_lowering=False, debug=False)


# --------------------------------------------------------------------------
# phase 1: conv + patchify + attention + cov
# --------------------------------------------------------------------------

def _build_phase1():
    import concourse.bass as bass
    import concourse.tile as tile
    from concourse import mybir
    from concourse.masks import make_identity

    f32 = mybir.dt.float32
    AF = mybir.ActivationFunctionType
    OP = mybir.AluOpType
    nc = _new_nc()

    xb_d = nc.dram_tensor("xb", (16, 258, 258), f32, kind="ExternalInput")
    w1_d = nc.dram_tensor("w1big", (128, 96), f32, kind="ExternalInput")
    wdw_d = nc.dram_tensor("wdwtap", (96, 9), f32, kind="ExternalInput")
    tmp_d = nc.dram_tensor("tempu", (1, 1), f32, kind="ExternalInput")
    vpd_d = nc.dram_tensor("vpd", (4, 256, 256), f32, kind="ExternalOutput")
    cov_d = nc.dram_tensor("cov", (4, 256, 256), f32, kind="ExternalOutput")

    with tile.TileContext(nc) as tc:
        with (
            tc.tile_pool(name="big", bufs=1) as big,
            tc.tile_pool(name="unit", bufs=4) as up,
            tc.tile_pool(name="psA", bufs=2, space="PSUM") as psA,   # att, conv
            tc.tile_pool(name="psT", bufs=4, space="PSUM") as psT,   # tr
        ):
            ident = big.tile([128, 128], f32, tag="ident")
            make_identity(nc, ident)

            tempb = big.tile([128, 1], f32, tag="tempb")
            nc.sync.dma_start(
                tempb,
                bass.AP(tensor=tmp_d.ap().tensor, offset=0, ap=[[0, 128], [1, 1]]),
            )
            tempn = big.tile([128, 1], f32, tag="tempn")
            nc.vector.tensor_scalar_mul(tempn, tempb, -1.0)

            w1 = big.tile([128, 96], f32, tag="w1")
            nc.sync.dma_start(w1, w1_d.ap())
            wdw = big.tile([96, 9], f32, tag="wdw")
            nc.sync.dma_start(wdw, wdw_d.ap())

            # ---- x load in 3 separate tiles so the 1x1 conv can start while
            # later row-chunks are still in flight. Tile k covers padded rows
            # [xsplit[k], xsplit[k+1]) of each band (chunk boundaries at even
            # rows so each conv chunk reads within one tile).
            xsplit = [0, 12, 24, 34]
            x_ts = []
            xengs = [nc.sync, nc.scalar, nc.gpsimd]
            for k in range(3):
                r0, r1 = xsplit[k], xsplit[k + 1]
                xt = big.tile([128, r1 - r0, 258], f32, tag=f"x{k}")
                half = (r1 - r0) // 2
                for h in range(2):
                    a, b = r0 + h * half, r0 + (h + 1) * half if h == 0 else r1
                    a, b = (r0, r0 + half) if h == 0 else (r0 + half, r1)
                    xengs[(2 * k + h) % 3].dma_start(
                        xt[:, a - r0:b - r0, :],
                        bass.AP(tensor=xb_d.ap().tensor, offset=a * 258,
                                ap=[[32 * 258, 8], [258 * 258, 16],
                                    [1, (b - a) * 258]]),
                    )
                x_ts.append(xt)

            # ---- 1x1 conv into padded q1 [96=(o12,band8), 34, 258]
            q1 = big.tile([96, 34, 258], f32, tag="q1")
            nc.vector.memset(q1[:, :, 0], 0.0)
            nc.vector.memset(q1[:, :, 257], 0.0)
            for ch in range(17):
                r0 = 2 * ch
                k = 0 if r0 < 12 else (1 if r0 < 24 else 2)
                xt = x_ts[k]
                rb = r0 - xsplit[k]
                acc = psA.tile([96, 512], f32, tag="conv")
                nc.tensor.matmul(acc, w1, xt[:, rb:rb + 2, 1:257],
                                 start=True, stop=True)
                if ch % 2 == 0:
                    nc.scalar.copy(
                        q1[:, r0:r0 + 2, 1:257],
                        acc.rearrange("p (a b) -> p a b", a=2))
                else:
                    nc.vector.tensor_copy(
                        q1[:, r0:r0 + 2, 1:257],
                        acc.rearrange("p (a b) -> p a b", a=2))

            # ---- depthwise 3x3: free-dim shifted FMA accumulation on DVE
            q2 = big.tile([96, 32, 256], f32, tag="q2")
            for t, (dy, dx) in enumerate(TAPS):
                srcq = q1[:, 1 + dy:33 + dy, 1 + dx:257 + dx]
                if t == 0:
                    nc.vector.tensor_scalar_mul(q2, srcq, wdw[:, 0:1])
                else:
                    nc.vector.scalar_tensor_tensor(
                        q2, srcq, wdw[:, t:t + 1], q2, op0=OP.mult, op1=OP.add)

            # ---- free-dim patchify permute (image -> patch order per band)
            q2p = big.tile([96, 8192], f32, tag="q2p")
            for r2h in range(2):
                nc.scalar.copy(
                    q2p[:, 4096 * r2h:4096 * (r2h + 1)].rearrange(
                        "p (c b d) -> p c b d", c=16, b=16),
                    q2[:, 16 * r2h:16 * (r2h + 1), :].rearrange(
                        "p b (c d) -> p c b d", c=16))

            # ---- patchified v -> DRAM (phase2 reads it contiguously)
            with nc.allow_non_contiguous_dma("patchify gathers"):
                nc.sync.dma_start(
                    bass.AP(tensor=vpd_d.ap().tensor, offset=0,
                            ap=[[8192, 32], [1, 8192]]),
                    q2p[64:96])

                # ---- per-unit pipeline, issued stage-by-stage across all 4
                # units so every engine queue stays fed
                U = 4
                q_pd, k_pd, qT, kT, xc, xcT = {}, {}, {}, {}, {}, {}
                scr, tnrq, covs = {}, {}, {}
                gi = 0
                for u in range(U):
                    q_pd[u] = up.tile([128, 2, 256], f32, tag="q_pd", name=f"q_pd{u}")
                    k_pd[u] = up.tile([128, 2, 256], f32, tag="k_pd", name=f"k_pd{u}")
                    for (osel, dst_pd) in ((u, q_pd[u]), (4 + u, k_pd[u])):
                        for c2 in range(2):
                            srcg = q2p[osel * 8 + 4 * c2:osel * 8 + 4 * c2 + 4]
                            geng = (nc.sync, nc.scalar, nc.gpsimd)[gi % 3]
                            gi += 1
                            geng.dma_start(
                                dst_pd[:, c2, :],
                                srcg.rearrange("p (a b) -> p a b", a=32))

                # row rsqrt norms; k scaled in place, q folded into zt
                for u in range(U):
                    scr[u] = up.tile([128, 256], f32, tag="scr", name=f"scr{u}")
                    rins = {}
                    for ti, t_pd in enumerate((q_pd[u], k_pd[u])):
                        nrm = up.tile([128, 2], f32, tag=f"nrm{ti}")
                        for c2 in range(2):
                            nc.scalar.activation(scr[u], t_pd[:, c2, :],
                                                 AF.Square,
                                                 accum_out=nrm[:, c2:c2 + 1])
                        nc.vector.tensor_scalar_max(nrm, nrm, 1e-24)
                        srt = up.tile([128, 2], f32, tag=f"srt{ti}")
                        nc.scalar.sqrt(srt, nrm)
                        rin = up.tile([128, 2], f32, tag=f"rin{ti}")
                        nc.vector.reciprocal(rin, srt)
                        nwt = up.tile([128, 2], f32, tag=f"nwt{ti}")
                        nc.vector.tensor_mul(nwt, nrm, rin)
                        nc.vector.tensor_mul(nwt, nwt, rin)
                        nc.vector.tensor_scalar(nwt, nwt, -0.5, 1.5,
                                                op0=OP.mult, op1=OP.add)
                        nc.vector.tensor_mul(rin, rin, nwt)
                        rins[ti] = rin
                    for c2 in range(2):
                        nc.scalar.activation(
                            k_pd[u][:, c2, :], k_pd[u][:, c2, :], AF.Copy,
                            scale=rins[1][:, c2:c2 + 1])
                    tnrq[u] = up.tile([128, 2], f32, tag="tnrq", name=f"tnrq{u}")
                    nc.vector.tensor_scalar_mul(tnrq[u], rins[0],
                                                tempn[:, 0:1])

                # exact f32 PE transposes -> qT,kT [d-part, dchunk, n]
                for u in range(U):
                    qT[u] = up.tile([128, 2, 256], f32, tag="qT", name=f"qT{u}")
                    kT[u] = up.tile([128, 2, 256], f32, tag="kT", name=f"kT{u}")
                    ti = 0
                    for (src_t, dst_t) in ((q_pd[u], qT[u]), (k_pd[u], kT[u])):
                        for pc in range(2):
                            for dc in range(2):
                                tp = psT.tile([128, 128], f32, tag="tr")
                                nc.tensor.transpose(
                                    tp, src_t[:, pc, 128 * dc:128 * (dc + 1)],
                                    ident)
                                if ti % 2 == 0:
                                    nc.scalar.copy(
                                        dst_t[:, dc, 128 * pc:128 * (pc + 1)],
                                        tp)
                                else:
                                    nc.vector.tensor_copy(
                                        dst_t[:, dc, 128 * pc:128 * (pc + 1)],
                                        tp)
                                ti += 1

                # attn + softmax (direct e^{-z} poly) -> xc, then xcT, cov
                for u in range(U):
                    xc[u] = up.tile([128, 2, 256], f32, tag="xc", name=f"xc{u}")
                    for nch in range(2):
                        att = psA.tile([128, 256], f32, tag="att")
                        for dc in range(2):
                            nc.tensor.matmul(
                                att, qT[u][:, dc, 128 * nch:128 * (nch + 1)],
                                kT[u][:, dc, :], start=(dc == 0),
                                stop=(dc == 1))
                        zt = up.tile([128, 256], f32, tag="zt")
                        nc.vector.tensor_scalar(zt, att,
                                                tnrq[u][:, nch:nch + 1],
                                                tempb[:, 0:1],
                                                op0=OP.mult, op1=OP.add)
                        ep = up.tile([128, 256], f32, tag="ep")
                        nc.vector.tensor_scalar_mul(ep, zt, NEGEXP[10])
                        for kk in range(9, 0, -1):
                            nc.vector.scalar_tensor_tensor(
                                ep, ep, NEGEXP[kk], zt, op0=OP.add,
                                op1=OP.mult)
                        nc.vector.tensor_scalar_add(ep, ep, NEGEXP[0])
                        rssum = up.tile([128, 1], f32, tag="rssum")
                        nc.scalar.activation(scr[u], ep, AF.Copy,
                                             accum_out=rssum)
                        rowsum = up.tile([128, 1], f32, tag="rowsum")
                        nc.vector.reciprocal(rowsum, rssum)
                        nwt2 = up.tile([128, 1], f32, tag="nwt2")
                        nc.vector.tensor_mul(nwt2, rssum, rowsum)
                        nc.vector.tensor_scalar(nwt2, nwt2, -1.0, 2.0,
                                                op0=OP.mult, op1=OP.add)
                        nc.vector.tensor_mul(rowsum, rowsum, nwt2)
                        nc.vector.tensor_scalar(xc[u][:, nch, :], ep,
                                                rowsum[:, 0:1], 1.0 / 256.0,
                                                op0=OP.mult, op1=OP.subtract)

                for u in range(U):
                    xcT[u] = up.tile([128, 2, 256], f32, tag="xcT", name=f"xcT{u}")
                    for pc in range(2):
                        for dc in range(2):
                            tp = psT.tile([128, 128], f32, tag="tr")
                            nc.tensor.transpose(
                                tp, xc[u][:, pc, 128 * dc:128 * (dc + 1)],
                                ident)
                            if (pc + dc) % 2 == 0:
                                nc.scalar.copy(
                                    xcT[u][:, dc, 128 * pc:128 * (pc + 1)],
                                    tp)
                            else:
                                nc.vector.tensor_copy(
                                    xcT[u][:, dc, 128 * pc:128 * (pc + 1)],
                                    tp)

                # raw S = xc xc^T; /trace + eps I happen on the host in f64
                for u in range(U):
                    covs[u] = up.tile([128, 2, 256], f32, tag="cov_sb", name=f"cov_sb{u}")
                    for nch in range(2):
                        cv = psA.tile([128, 256], f32, tag="att")
                        for mc in range(2):
                            nc.tensor.matmul(
                                cv, xcT[u][:, mc, 128 * nch:128 * (nch + 1)],
                                xcT[u][:, mc, :], start=(mc == 0),
                                stop=(mc == 1))
                        if nch == 0:
                            nc.scalar.copy(covs[u][:, nch, :], cv)
                        else:
                            nc.vector.tensor_copy(covs[u][:, nch, :], cv)
                    (nc.sync, nc.scalar, nc.gpsimd, nc.sync)[u].dma_start(
                        cov_d.ap()[u].rearrange("(c p) m -> p c m", p=128),
                        covs[u])

    nc.compile()
    return nc


# --------------------------------------------------------------------------
# phase 2+3 merged: Y = w_fr U^T, A^T slice, M slice, 1x1 conv out (f32r)
# per-core = (batch, 64-patch slice of n); host supplies full U^T of the
# batch plus the 64-column slice of it.
# --------------------------------------------------------------------------

def _build_phase23(conv_f32r=True):
    import concourse.bass as bass
    import concourse.tile as tile
    from concourse import mybir

    f32 = mybir.dt.float32
    f32r = mybir.dt.float32r
    mdt = f32r if conv_f32r else f32
    nc = _new_nc()

    ut_d = nc.dram_tensor("ut", (16, 100, 256), f32, kind="ExternalInput")
    utsl_d = nc.dram_tensor("utsl", (16, 100, 64), f32, kind="ExternalInput")
    vall_d = nc.dram_tensor("vall", (16, 256, 256), f32, kind="ExternalInput")
    wfrT_d = nc.dram_tensor("wfrT", (100, 100), f32, kind="ExternalInput")
    wpoT_d = nc.dram_tensor("wpoT", (16, 16), f32, kind="ExternalInput")
    outq_d = nc.dram_tensor("outq", (16, 64, 256), f32, kind="ExternalOutput")
    mb_d = nc.dram_tensor("mb", (16, 64, 256), f32, kind="Internal")

    with tile.TileContext(nc) as tc:
        with (
            tc.tile_pool(name="sb", bufs=1) as sb,
            tc.tile_pool(name="unit", bufs=2) as up,
            tc.tile_pool(name="ps", bufs=2, space="PSUM") as ps,    # yt, mp, op
            tc.tile_pool(name="ps1", bufs=1, space="PSUM") as ps1,  # ytn, at
        ):
            wfrT = sb.tile([100, 100], f32, tag="wfrT")
            nc.sync.dma_start(wfrT, wfrT_d.ap())
            wpoT = sb.tile([16, 16], f32, tag="wpoT")
            nc.sync.dma_start(wpoT, wpoT_d.ap())
            wpoTr = sb.tile([16, 16], mdt, tag="wpoTr")
            nc.vector.tensor_copy(wpoTr, wpoT)

            # all-units U^T loads: [100, 16u, 256] and slice [100, 16u, 64]
            ut_sb = sb.tile([100, 16, 256], f32, tag="ut_sb")
            for uh in range(4):
                (nc.sync, nc.scalar, nc.gpsimd, nc.sync)[uh].dma_start(
                    ut_sb[:, 4 * uh:4 * uh + 4, :],
                    bass.AP(tensor=ut_d.ap().tensor, offset=uh * 4 * 25600,
                            ap=[[256, 100], [25600, 4], [1, 256]]))
            utsl_sb = sb.tile([100, 16, 64], f32, tag="utsl_sb")
            nc.scalar.dma_start(
                utsl_sb,
                bass.AP(tensor=utsl_d.ap().tensor, offset=0,
                        ap=[[64, 100], [6400, 16], [1, 64]]))

            # Y^T for all units, batched: [100, 16u, 256] (shared stationary)
            yt_all = sb.tile([100, 16, 256], f32, tag="yt_all")
            for bh in range(8):
                ytp = ps.tile([100, 512], f32, tag="yt")
                nc.tensor.matmul(ytp, wfrT,
                                 ut_sb.rearrange("p a b -> p (a b)")
                                 [:, 512 * bh:512 * (bh + 1)],
                                 start=True, stop=True)
                (nc.scalar.copy if bh % 2 == 0 else nc.vector.tensor_copy)(
                    yt_all.rearrange("p a b -> p (a b)")
                    [:, 512 * bh:512 * (bh + 1)], ytp)
            ytn_all = sb.tile([100, 16, 64], f32, tag="ytn_all")
            for bh in range(2):
                ynp = ps1.tile([100, 512], f32, tag="ytn")
                nc.tensor.matmul(ynp, wfrT,
                                 utsl_sb.rearrange("p a b -> p (a b)")
                                 [:, 512 * bh:512 * (bh + 1)],
                                 start=True, stop=True)
                (nc.scalar.copy if bh == 0 else nc.vector.tensor_copy)(
                    ytn_all.rearrange("p a b -> p (a b)")
                    [:, 512 * bh:512 * (bh + 1)], ynp)

            # mstack: [64 part(n-local), 16 c, 256 d]
            mstack = sb.tile([64, 16, 256], f32, tag="mstack")

            for u in range(16):
                # per-unit v load + f32r rounding
                v_u = up.tile([128, 2, 256], f32, tag="v_u")
                (nc.sync, nc.scalar, nc.gpsimd)[u % 3].dma_start(
                    v_u,
                    bass.AP(tensor=vall_d.ap().tensor, offset=u * 65536,
                            ap=[[256, 128], [32768, 2], [1, 256]]))
                vr_u = up.tile([128, 2, 256], mdt, tag="vr_u")
                if u % 2 == 0:
                    nc.scalar.copy(
                        vr_u.rearrange("p a b -> p (a b)"),
                        v_u.rearrange("p a b -> p (a b)"))
                else:
                    nc.vector.tensor_copy(
                        vr_u.rearrange("p a b -> p (a b)"),
                        v_u.rearrange("p a b -> p (a b)"))

                # A^T slice: [m 2x128, n 64]
                at_sb = up.tile([128, 2, 64], mdt, tag="at_sb")
                for mc in range(2):
                    atp = ps1.tile([128, 64], f32, tag="at")
                    nc.tensor.matmul(atp,
                                     yt_all[:, u, 128 * mc:128 * (mc + 1)],
                                     ytn_all[:, u, :], start=True, stop=True)
                    (nc.scalar.copy if mc == 0 else nc.vector.tensor_copy)(
                        at_sb[:, mc, :], atp)

                # M slice: [n 64, d 256] = A[n,:] @ v  (lhsT = A^T chunks)
                mp = ps.tile([64, 256], f32, tag="mp")
                for mc in range(2):
                    nc.tensor.matmul(mp, at_sb[:, mc, :], vr_u[:, mc, :],
                                     start=(mc == 0), stop=(mc == 1))
                (nc.scalar.copy if u % 2 == 0 else nc.vector.tensor_copy)(
                    mstack[:, u, :], mp)

            # bounce M through DRAM to move channels onto partitions
            # mb layout: [c, n_local, d]
            nc.sync.dma_start(
                bass.AP(tensor=mb_d.ap().tensor, offset=0,
                        ap=[[256, 64], [16384, 16], [1, 256]]),
                mstack)
            # conv-ready in 2 halves; the (bq,r)->(r,bq) patchify interleave
            # rides the f32r-rounding engine copy, so all DMAs are contiguous
            engs = ["scalar", "vector"]
            for half in range(2):
                m_raw = sb.tile([16, 8192], f32, tag="m_raw")
                nc.sync.dma_start(
                    m_raw,
                    bass.AP(tensor=mb_d.ap().tensor, offset=half * 8192,
                            ap=[[16384, 16], [1, 8192]]))
                m_r = sb.tile([16, 8192], mdt, tag="m_r")
                cpeng = nc.scalar if half == 0 else nc.vector
                for al in range(2):
                    dst_v = m_r[:, 4096 * al:4096 * (al + 1)].rearrange(
                        "p (b c d) -> p b c d", b=16, c=16)
                    src_v = m_raw[:, 4096 * al:4096 * (al + 1)].rearrange(
                        "p (c b d) -> p b c d", c=16, b=16)
                    if cpeng is nc.scalar:
                        cpeng.copy(dst_v, src_v)
                    else:
                        cpeng.tensor_copy(dst_v, src_v)
                o_sb = sb.tile([16, 8192], f32, tag="o_sb")
                for ch in range(16):
                    op = ps.tile([16, 512], f32, tag="op")
                    nc.tensor.matmul(op, wpoTr,
                                     m_r[:, 512 * ch:512 * (ch + 1)],
                                     start=True, stop=True)
                    eng = getattr(nc, engs[ch % 2])
                    if eng is nc.scalar:
                        eng.copy(o_sb[:, 512 * ch:512 * (ch + 1)], op)
                    else:
                        eng.tensor_copy(o_sb[:, 512 * ch:512 * (ch + 1)], op)
                nc.sync.dma_start(
                    bass.AP(tensor=outq_d.ap().tensor, offset=half * 8192,
                            ap=[[16384, 16], [1, 8192]]),
                    o_sb)

    nc.compile()
    return nc


# --------------------------------------------------------------------------
# host orchestration
# --------------------------------------------------------------------------

def _get(name):
    if name not in _built:
        if name == "p1":
            _built[name] = _build_phase1()
        else:
            _built[name] = _build_phase23(conv_f32r=True)
    return _built[name]


def make_phase1_inputs(x, w_qkv, w_dw, temperature):
    xpads = []
    for b in range(B):
        xpad = np.zeros((16, 258, 258), np.float32)
        xpad[:, 1:257, 1:257] = x[b]
        xpads.append(xpad)
    ins = []
    for k in range(NCORES):
        b, g = divmod(k, 4)
        rows = ([4 * g + u for u in range(4)]
                + [16 + 4 * g + u for u in range(4)]
                + [32 + 4 * g + u for u in range(4)])
        w1big = np.zeros((128, 96), np.float32)
        wdwtap = np.zeros((96, 9), np.float32)
        for bb in range(8):
            for o in range(12):
                w1big[bb * 16:(bb + 1) * 16, o * 8 + bb] = w_qkv[rows[o]]
                wdwtap[o * 8 + bb] = w_dw[rows[o], 0].reshape(9)
        ins.append({
            "xb": xpads[b],
            "w1big": w1big,
            "wdwtap": wdwtap,
            "tempu": np.full((1, 1), temperature[g, 0, 0], np.float32),
        })
    return ins


def _host_eigh(cov_all):
    """cov_all: raw S (32,256,256) f32 -> top-100 eigvecs of
    S/trace(S) + 1e-5 I via jax CPU f64 eigh."""
    import jax
    jax.config.update("jax_enable_x64", True)
    import jax.numpy as jnp
    cpu = jax.devices("cpu")[0]
    s = cov_all.astype(np.float64)
    tra = np.trace(s, axis1=-2, axis2=-1)[:, None, None]
    s = s / tra + 1e-5 * np.eye(256, dtype=np.float64)
    with jax.default_device(cpu):
        _, vecs = jnp.linalg.eigh(jnp.asarray(s))
        U = np.asarray(vecs)[:, :, ::-1][:, :, :TOP_P]
    return U


def _run(name, nc, in_maps):
    from concourse.bass_utils import run_bass_kernel_spmd
    r = run_bass_kernel_spmd(nc, in_maps, core_ids=list(range(NCORES)),
                             trace=PROFILE)
    if PROFILE:
        LAST_PROFILE.append((name, r))
    return r.results


def kernel(x, w_qkv, w_dw, temperature, w_fr, w_po):
    x = np.ascontiguousarray(np.asarray(x, dtype=np.float32))
    w_qkv = np.asarray(w_qkv, dtype=np.float32)
    w_dw = np.asarray(w_dw, dtype=np.float32)
    temperature = np.asarray(temperature, dtype=np.float32)
    w_fr = np.asarray(w_fr, dtype=np.float32)
    w_po = np.asarray(w_po, dtype=np.float32)

    # ---- phase 1
    nc1 = _get("p1")
    in1 = make_phase1_inputs(x, w_qkv, w_dw, temperature)
    res1 = _run("p1", nc1, in1)

    # ---- host eigh
    cov_all = np.zeros((B, C, 256, 256), np.float32)
    vfull = np.zeros((B, C, 256, 256), np.float32)
    for k in range(NCORES):
        b, g = divmod(k, 4)
        cov_all[b, 4 * g:4 * g + 4] = res1[k]["cov"]
        vfull[b, 4 * g:4 * g + 4] = res1[k]["vpd"]
    DEBUG["cov_all"] = cov_all
    DEBUG["vfull"] = vfull
    U = _host_eigh(cov_all.reshape(-1, 256, 256))
    UT = np.ascontiguousarray(
        U.transpose(0, 2, 1).astype(np.float32)).reshape(B, C, TOP_P, 256)

    # ---- phase 2+3
    nc23 = _get("p23")
    wfrT = np.ascontiguousarray(w_fr.T)
    wpoT = np.ascontiguousarray(w_po.T)
    in23 = []
    for k in range(NCORES):
        b, qr = divmod(k, 4)
        in23.append({
            "ut": UT[b],
            "utsl": np.ascontiguousarray(UT[b][:, :, 64 * qr:64 * (qr + 1)]),
            "vall": vfull[b],
            "wfrT": wfrT,
            "wpoT": wpoT,
        })
    try:
        res23 = _run("p23", nc23, in23)
    except Exception:
        # f32r DMA provenance may be rejected by the BIR verifier at
        # NEFF-compile time; fall back to a plain-f32 conv build.
        _built["p23"] = _build_phase23(conv_f32r=False)
        res23 = _run("p23", _built["p23"], in23)

    out = np.zeros((B, C, 256, 256), np.float32)
    for k in range(NCORES):
        b, qr = divmod(k, 4)
        out[b, :, 64 * qr:64 * (qr + 1), :] = res23[k]["outq"]
    return out


# revision 38
# speedup vs baseline: 1.2421x; 1.0463x over previous
"""Trainium2 Bass kernel for nn_AttentionSpatial (manifold attention).

Pipeline (per the reference):
  qkv = 1x1 conv -> 3x3 depthwise conv -> patchify -> per-(b,head,c) unit:
  normalize q,k -> attn = softmax(q k^T * temp) -> SPD cov -> eigh (top-100)
  -> A = U (w_fr^T w_fr) U^T -> out = A v -> re-patchify -> 1x1 conv out.

The eigendecomposition runs on the host via jax CPU float64 eigh (LAPACK
sign conventions make any on-device eigensolver unusable), and the
attention/cov pipeline upstream of it must stay f32-exact: the eigh
amplifies cov perturbations ~1e4x, so bf16/f32r/table-exp anywhere before
cov blows past the error gate.  Downstream of the eigh (phase 2) f32r is
safe.

Two device launches:
  phase1 (per-core = batch x 4-channel group): 1x1 conv (PE f32) ->
    depthwise 3x3 (DVE shifted FMAs) -> patchify gathers -> normalize ->
    exact f32 PE transposes -> attn -> softmax via direct e^{-z} poly
    (DVE/Pool split) -> centered cov.  Outputs cov + patchified v.
  phase23 (per-core = batch x 64-patch slice): Y = w_fr U^T, A^T slices,
    M = A v, channel-mixing 1x1 conv in f32r, fully contiguous output.
"""

import numpy as np

PATCH = 16
HEADS = 4
TOP_P = 100
B, C = 2, 16
NCORES = 8

_built = {}
PROFILE = False
LAST_PROFILE = []
DEBUG = {}

# direct minimax fit of e^{-z} on z in [0,2] (degree 10, f32 Horner)
NEGEXP = [0.9999999999715448, -0.9999999981035824, 0.4999999688617123,
          -0.16666644598277386, 0.041665826247535086, -0.00833141300995797,
          0.0013860949436704422, -0.00019574854948869083,
          2.3127409315526552e-05, -2.0742207592879798e-06,
          1.0360396033546386e-07]

TAPS = [(dy, dx) for dy in (-1, 0, 1) for dx in (-1, 0, 1)]


def _new_nc():
    from concourse import bacc
    return bacc.Bacc("TRN2", target_bir_lowering=False, debug=False)


# --------------------------------------------------------------------------
# phase 1: conv + patchify + attention + cov
# --------------------------------------------------------------------------

def _build_phase1():
    import concourse.bass as bass
    import concourse.tile as tile
    from concourse import mybir
    from concourse.masks import make_identity

    f32 = mybir.dt.float32
    AF = mybir.ActivationFunctionType
    OP = mybir.AluOpType
    nc = _new_nc()

    xb_d = nc.dram_tensor("xb", (16, 258, 258), f32, kind="ExternalInput")
    w1_d = nc.dram_tensor("w1big", (128, 96), f32, kind="ExternalInput")
    wdw_d = nc.dram_tensor("wdwtap", (96, 9), f32, kind="ExternalInput")
    tmp_d = nc.dram_tensor("tempu", (1, 1), f32, kind="ExternalInput")
    vpd_d = nc.dram_tensor("vpd", (4, 256, 256), f32, kind="ExternalOutput")
    cov_d = nc.dram_tensor("cov", (4, 256, 256), f32, kind="ExternalOutput")

    with tile.TileContext(nc) as tc:
        with (
            tc.tile_pool(name="big", bufs=1) as big,
            tc.tile_pool(name="unit", bufs=4) as up,
            tc.tile_pool(name="psA", bufs=2, space="PSUM") as psA,   # att, conv
            tc.tile_pool(name="psT", bufs=4, space="PSUM") as psT,   # tr
        ):
            ident = big.tile([128, 128], f32, tag="ident")
            make_identity(nc, ident)

            tempb = big.tile([128, 1], f32, tag="tempb")
            nc.sync.dma_start(
                tempb,
                bass.AP(tensor=tmp_d.ap().tensor, offset=0, ap=[[0, 128], [1, 1]]),
            )
            tempn = big.tile([128, 1], f32, tag="tempn")
            nc.vector.tensor_scalar_mul(tempn, tempb, -1.0)

            w1 = big.tile([128, 96], f32, tag="w1")
            nc.sync.dma_start(w1, w1_d.ap())
            wdw = big.tile([96, 9], f32, tag="wdw")
            nc.sync.dma_start(wdw, wdw_d.ap())

            # ---- x load in 3 separate tiles so the 1x1 conv can start while
            # later row-chunks are still in flight. Tile k covers padded rows
            # [xsplit[k], xsplit[k+1]) of each band (chunk boundaries at even
            # rows so each conv chunk reads within one tile).
            xsplit = [0, 12, 24, 34]
            x_ts = []
            xengs = [nc.sync, nc.scalar, nc.gpsimd]
            for k in range(3):
                r0, r1 = xsplit[k], xsplit[k + 1]
                xt = big.tile([128, r1 - r0, 258], f32, tag=f"x{k}")
                half = (r1 - r0) // 2
                for h in range(2):
                    a, b = r0 + h * half, r0 + (h + 1) * half if h == 0 else r1
                    a, b = (r0, r0 + half) if h == 0 else (r0 + half, r1)
                    xengs[(2 * k + h) % 3].dma_start(
                        xt[:, a - r0:b - r0, :],
                        bass.AP(tensor=xb_d.ap().tensor, offset=a * 258,
                                ap=[[32 * 258, 8], [258 * 258, 16],
                                    [1, (b - a) * 258]]),
                    )
                x_ts.append(xt)

            # ---- 1x1 conv into padded q1 [96=(o12,band8), 34, 258]
            q1 = big.tile([96, 34, 258], f32, tag="q1")
            nc.vector.memset(q1[:, :, 0], 0.0)
            nc.vector.memset(q1[:, :, 257], 0.0)
            for ch in range(17):
                r0 = 2 * ch
                k = 0 if r0 < 12 else (1 if r0 < 24 else 2)
                xt = x_ts[k]
                rb = r0 - xsplit[k]
                acc = psA.tile([96, 512], f32, tag="conv")
                nc.tensor.matmul(acc, w1, xt[:, rb:rb + 2, 1:257],
                                 start=True, stop=True)
                if ch % 2 == 0:
                    nc.scalar.copy(
                        q1[:, r0:r0 + 2, 1:257],
                        acc.rearrange("p (a b) -> p a b", a=2))
                else:
                    nc.vector.tensor_copy(
                        q1[:, r0:r0 + 2, 1:257],
                        acc.rearrange("p (a b) -> p a b", a=2))

            # ---- depthwise 3x3: free-dim shifted FMA accumulation on DVE
            q2 = big.tile([96, 32, 256], f32, tag="q2")
            for t, (dy, dx) in enumerate(TAPS):
                srcq = q1[:, 1 + dy:33 + dy, 1 + dx:257 + dx]
                if t == 0:
                    nc.vector.tensor_scalar_mul(q2, srcq, wdw[:, 0:1])
                else:
                    nc.vector.scalar_tensor_tensor(
                        q2, srcq, wdw[:, t:t + 1], q2, op0=OP.mult, op1=OP.add)

            # ---- free-dim patchify permute (image -> patch order per band)
            q2p = big.tile([96, 8192], f32, tag="q2p")
            for r2h in range(2):
                dstv = q2p[:, 4096 * r2h:4096 * (r2h + 1)].rearrange(
                    "p (c b d) -> p c b d", c=16, b=16)
                srcv = q2[:, 16 * r2h:16 * (r2h + 1), :].rearrange(
                    "p b (c d) -> p c b d", c=16)
                if r2h == 0:
                    nc.scalar.copy(dstv, srcv)
                else:
                    nc.vector.tensor_copy(dstv, srcv)

            with nc.allow_non_contiguous_dma("patchify gathers"):
                # ---- per-unit pipeline, issued stage-by-stage across all 4
                # units so every engine queue stays fed
                U = 4
                q_pd, k_pd, qT, kT, xc, xcT = {}, {}, {}, {}, {}, {}
                scr, tnrq, covs = {}, {}, {}
                gi = 0
                for u in range(U):
                    q_pd[u] = up.tile([128, 2, 256], f32, tag="q_pd", name=f"q_pd{u}")
                    k_pd[u] = up.tile([128, 2, 256], f32, tag="k_pd", name=f"k_pd{u}")
                    for (osel, dst_pd) in ((u, q_pd[u]), (4 + u, k_pd[u])):
                        for c2 in range(2):
                            srcg = q2p[osel * 8 + 4 * c2:osel * 8 + 4 * c2 + 4]
                            geng = (nc.sync, nc.scalar, nc.gpsimd)[gi % 3]
                            gi += 1
                            geng.dma_start(
                                dst_pd[:, c2, :],
                                srcg.rearrange("p (a b) -> p a b", a=32))

                # ---- patchified v -> DRAM (phase2 reads it contiguously)
                nc.sync.dma_start(
                    bass.AP(tensor=vpd_d.ap().tensor, offset=0,
                            ap=[[8192, 32], [1, 8192]]),
                    q2p[64:96])

                # row rsqrt norms; k scaled in place, q folded into zt
                for u in range(U):
                    scr[u] = up.tile([128, 256], f32, tag="scr", name=f"scr{u}")
                    rins = {}
                    for ti, t_pd in enumerate((q_pd[u], k_pd[u])):
                        nrm = up.tile([128, 2], f32, tag=f"nrm{ti}")
                        for c2 in range(2):
                            nc.scalar.activation(scr[u], t_pd[:, c2, :],
                                                 AF.Square,
                                                 accum_out=nrm[:, c2:c2 + 1])
                        nc.vector.tensor_scalar_max(nrm, nrm, 1e-24)
                        srt = up.tile([128, 2], f32, tag=f"srt{ti}")
                        nc.scalar.sqrt(srt, nrm)
                        rin = up.tile([128, 2], f32, tag=f"rin{ti}")
                        nc.vector.reciprocal(rin, srt)
                        nwt = up.tile([128, 2], f32, tag=f"nwt{ti}")
                        nc.vector.tensor_mul(nwt, nrm, rin)
                        nc.vector.tensor_mul(nwt, nwt, rin)
                        nc.vector.tensor_scalar(nwt, nwt, -0.5, 1.5,
                                                op0=OP.mult, op1=OP.add)
                        nc.vector.tensor_mul(rin, rin, nwt)
                        rins[ti] = rin
                    for c2 in range(2):
                        nc.scalar.activation(
                            k_pd[u][:, c2, :], k_pd[u][:, c2, :], AF.Copy,
                            scale=rins[1][:, c2:c2 + 1])
                    tnrq[u] = up.tile([128, 2], f32, tag="tnrq", name=f"tnrq{u}")
                    nc.vector.tensor_scalar_mul(tnrq[u], rins[0],
                                                tempn[:, 0:1])

                # exact f32 PE transposes -> qT,kT [d-part, dchunk, n]
                for u in range(U):
                    qT[u] = up.tile([128, 2, 256], f32, tag="qT", name=f"qT{u}")
                    kT[u] = up.tile([128, 2, 256], f32, tag="kT", name=f"kT{u}")
                    ti = 0
                    for (src_t, dst_t) in ((q_pd[u], qT[u]), (k_pd[u], kT[u])):
                        for pc in range(2):
                            for dc in range(2):
                                tp = psT.tile([128, 128], f32, tag="tr")
                                nc.tensor.transpose(
                                    tp, src_t[:, pc, 128 * dc:128 * (dc + 1)],
                                    ident)
                                if ti % 2 == 0:
                                    nc.scalar.copy(
                                        dst_t[:, dc, 128 * pc:128 * (pc + 1)],
                                        tp)
                                else:
                                    nc.vector.tensor_copy(
                                        dst_t[:, dc, 128 * pc:128 * (pc + 1)],
                                        tp)
                                ti += 1

                # attn + softmax (direct e^{-z} poly) -> xc, then xcT, cov
                for u in range(U):
                    xc[u] = up.tile([128, 2, 256], f32, tag="xc", name=f"xc{u}")
                    for nch in range(2):
                        att = psA.tile([128, 256], f32, tag="att")
                        for dc in range(2):
                            nc.tensor.matmul(
                                att, qT[u][:, dc, 128 * nch:128 * (nch + 1)],
                                kT[u][:, dc, :], start=(dc == 0),
                                stop=(dc == 1))
                        zt = up.tile([128, 256], f32, tag="zt")
                        nc.vector.tensor_scalar(zt, att,
                                                tnrq[u][:, nch:nch + 1],
                                                tempb[:, 0:1],
                                                op0=OP.mult, op1=OP.add)
                        ep = up.tile([128, 256], f32, tag="ep")
                        nc.vector.tensor_scalar_mul(ep, zt, NEGEXP[10])
                        for kk in range(9, 0, -1):
                            nc.vector.scalar_tensor_tensor(
                                ep, ep, NEGEXP[kk], zt, op0=OP.add,
                                op1=OP.mult)
                        nc.vector.tensor_scalar_add(ep, ep, NEGEXP[0])
                        rssum = up.tile([128, 1], f32, tag="rssum")
                        nc.scalar.activation(scr[u], ep, AF.Copy,
                                             accum_out=rssum)
                        rowsum = up.tile([128, 1], f32, tag="rowsum")
                        nc.vector.reciprocal(rowsum, rssum)
                        nwt2 = up.tile([128, 1], f32, tag="nwt2")
                        nc.vector.tensor_mul(nwt2, rssum, rowsum)
                        nc.vector.tensor_scalar(nwt2, nwt2, -1.0, 2.0,
                                                op0=OP.mult, op1=OP.add)
                        nc.vector.tensor_mul(rowsum, rowsum, nwt2)
                        nc.vector.tensor_scalar(xc[u][:, nch, :], ep,
                                                rowsum[:, 0:1], 1.0 / 256.0,
                                                op0=OP.mult, op1=OP.subtract)

                for u in range(U):
                    xcT[u] = up.tile([128, 2, 256], f32, tag="xcT", name=f"xcT{u}")
                    for pc in range(2):
                        for dc in range(2):
                            tp = psT.tile([128, 128], f32, tag="tr")
                            nc.tensor.transpose(
                                tp, xc[u][:, pc, 128 * dc:128 * (dc + 1)],
                                ident)
                            if (pc + dc) % 2 == 0:
                                nc.scalar.copy(
                                    xcT[u][:, dc, 128 * pc:128 * (pc + 1)],
                                    tp)
                            else:
                                nc.vector.tensor_copy(
                                    xcT[u][:, dc, 128 * pc:128 * (pc + 1)],
                                    tp)

                # raw S = xc xc^T; /trace + eps I happen on the host in f64
                for u in range(U):
                    covs[u] = up.tile([128, 2, 256], f32, tag="cov_sb", name=f"cov_sb{u}")
                    for nch in range(2):
                        cv = psA.tile([128, 256], f32, tag="att")
                        for mc in range(2):
                            nc.tensor.matmul(
                                cv, xcT[u][:, mc, 128 * nch:128 * (nch + 1)],
                                xcT[u][:, mc, :], start=(mc == 0),
                                stop=(mc == 1))
                        if nch == 0:
                            nc.scalar.copy(covs[u][:, nch, :], cv)
                        else:
                            nc.vector.tensor_copy(covs[u][:, nch, :], cv)
                    (nc.sync, nc.scalar, nc.gpsimd, nc.sync)[u].dma_start(
                        cov_d.ap()[u].rearrange("(c p) m -> p c m", p=128),
                        covs[u])

    nc.compile()
    return nc


# --------------------------------------------------------------------------
# phase 2+3 merged: Y = w_fr U^T, A^T slice, M slice, 1x1 conv out (f32r)
# per-core = (batch, 64-patch slice of n); host supplies full U^T of the
# batch plus the 64-column slice of it.
# --------------------------------------------------------------------------

def _build_phase23(conv_f32r=True):
    import concourse.bass as bass
    import concourse.tile as tile
    from concourse import mybir

    f32 = mybir.dt.float32
    f32r = mybir.dt.float32r
    mdt = f32r if conv_f32r else f32
    nc = _new_nc()

    ut_d = nc.dram_tensor("ut", (16, 100, 256), f32, kind="ExternalInput")
    utsl_d = nc.dram_tensor("utsl", (16, 100, 64), f32, kind="ExternalInput")
    vall_d = nc.dram_tensor("vall", (16, 256, 256), f32, kind="ExternalInput")
    wfrT_d = nc.dram_tensor("wfrT", (100, 100), f32, kind="ExternalInput")
    wpoT_d = nc.dram_tensor("wpoT", (16, 16), f32, kind="ExternalInput")
    outq_d = nc.dram_tensor("outq", (16, 64, 256), f32, kind="ExternalOutput")
    mb_d = nc.dram_tensor("mb", (16, 64, 256), f32, kind="Internal")

    with tile.TileContext(nc) as tc:
        with (
            tc.tile_pool(name="sb", bufs=1) as sb,
            tc.tile_pool(name="unit", bufs=3) as up,
            tc.tile_pool(name="ps", bufs=2, space="PSUM") as ps,    # yt, mp, op
            tc.tile_pool(name="ps1", bufs=1, space="PSUM") as ps1,  # ytn, at
        ):
            wfrT = sb.tile([100, 100], f32, tag="wfrT")
            nc.sync.dma_start(wfrT, wfrT_d.ap())
            wpoT = sb.tile([16, 16], f32, tag="wpoT")
            nc.sync.dma_start(wpoT, wpoT_d.ap())
            wpoTr = sb.tile([16, 16], mdt, tag="wpoTr")
            nc.vector.tensor_copy(wpoTr, wpoT)

            # all-units U^T loads: [100, 16u, 256] and slice [100, 16u, 64]
            ut_sb = sb.tile([100, 16, 256], f32, tag="ut_sb")
            for uh in range(4):
                (nc.sync, nc.scalar, nc.gpsimd, nc.sync)[uh].dma_start(
                    ut_sb[:, 4 * uh:4 * uh + 4, :],
                    bass.AP(tensor=ut_d.ap().tensor, offset=uh * 4 * 25600,
                            ap=[[256, 100], [25600, 4], [1, 256]]))
            utsl_sb = sb.tile([100, 16, 64], f32, tag="utsl_sb")
            nc.scalar.dma_start(
                utsl_sb,
                bass.AP(tensor=utsl_d.ap().tensor, offset=0,
                        ap=[[64, 100], [6400, 16], [1, 64]]))

            # Y^T for all units, batched: [100, 16u, 256] (shared stationary)
            yt_all = sb.tile([100, 16, 256], f32, tag="yt_all")
            for bh in range(8):
                ytp = ps.tile([100, 512], f32, tag="yt")
                nc.tensor.matmul(ytp, wfrT,
                                 ut_sb.rearrange("p a b -> p (a b)")
                                 [:, 512 * bh:512 * (bh + 1)],
                                 start=True, stop=True)
                (nc.scalar.copy if bh % 2 == 0 else nc.vector.tensor_copy)(
                    yt_all.rearrange("p a b -> p (a b)")
                    [:, 512 * bh:512 * (bh + 1)], ytp)
            ytn_all = sb.tile([100, 16, 64], f32, tag="ytn_all")
            for bh in range(2):
                ynp = ps1.tile([100, 512], f32, tag="ytn")
                nc.tensor.matmul(ynp, wfrT,
                                 utsl_sb.rearrange("p a b -> p (a b)")
                                 [:, 512 * bh:512 * (bh + 1)],
                                 start=True, stop=True)
                (nc.scalar.copy if bh == 0 else nc.vector.tensor_copy)(
                    ytn_all.rearrange("p a b -> p (a b)")
                    [:, 512 * bh:512 * (bh + 1)], ynp)

            # mstack: [64 part(n-local), 16 c, 256 d]
            mstack = sb.tile([64, 16, 256], f32, tag="mstack")

            for u in range(16):
                # per-unit v load + f32r rounding
                v_u = up.tile([128, 2, 256], f32, tag="v_u")
                (nc.sync, nc.scalar, nc.gpsimd)[u % 3].dma_start(
                    v_u,
                    bass.AP(tensor=vall_d.ap().tensor, offset=u * 65536,
                            ap=[[256, 128], [32768, 2], [1, 256]]))
                vr_u = up.tile([128, 2, 256], mdt, tag="vr_u")
                if u % 2 == 0:
                    nc.scalar.copy(
                        vr_u.rearrange("p a b -> p (a b)"),
                        v_u.rearrange("p a b -> p (a b)"))
                else:
                    nc.vector.tensor_copy(
                        vr_u.rearrange("p a b -> p (a b)"),
                        v_u.rearrange("p a b -> p (a b)"))

                # A^T slice: [m 2x128, n 64]
                at_sb = up.tile([128, 2, 64], mdt, tag="at_sb")
                atp = ps1.tile([128, 2, 64], f32, tag="at")
                for mc in range(2):
                    nc.tensor.matmul(atp[:, mc, :],
                                     yt_all[:, u, 128 * mc:128 * (mc + 1)],
                                     ytn_all[:, u, :], start=True, stop=True)
                (nc.scalar.copy if u % 2 == 0 else nc.vector.tensor_copy)(
                    at_sb.rearrange("p a b -> p (a b)"),
                    atp.rearrange("p a b -> p (a b)"))

                # M slice: [n 64, d 256] = A[n,:] @ v  (lhsT = A^T chunks)
                mp = ps.tile([64, 256], f32, tag="mp")
                for mc in range(2):
                    nc.tensor.matmul(mp, at_sb[:, mc, :], vr_u[:, mc, :],
                                     start=(mc == 0), stop=(mc == 1))
                (nc.scalar.copy if u % 2 == 0 else nc.vector.tensor_copy)(
                    mstack[:, u, :], mp)

            # bounce M through DRAM to move channels onto partitions
            # mb layout: [c, n_local, d]
            nc.sync.dma_start(
                bass.AP(tensor=mb_d.ap().tensor, offset=0,
                        ap=[[256, 64], [16384, 16], [1, 256]]),
                mstack)
            # conv-ready in 2 halves; the (bq,r)->(r,bq) patchify interleave
            # rides the f32r-rounding engine copy, so all DMAs are contiguous
            engs = ["scalar", "vector"]
            for half in range(2):
                m_raw = sb.tile([16, 8192], f32, tag="m_raw")
                nc.sync.dma_start(
                    m_raw,
                    bass.AP(tensor=mb_d.ap().tensor, offset=half * 8192,
                            ap=[[16384, 16], [1, 8192]]))
                m_r = sb.tile([16, 8192], mdt, tag="m_r")
                cpeng = nc.scalar if half == 0 else nc.vector
                for al in range(2):
                    dst_v = m_r[:, 4096 * al:4096 * (al + 1)].rearrange(
                        "p (b c d) -> p b c d", b=16, c=16)
                    src_v = m_raw[:, 4096 * al:4096 * (al + 1)].rearrange(
                        "p (c b d) -> p b c d", c=16, b=16)
                    if cpeng is nc.scalar:
                        cpeng.copy(dst_v, src_v)
                    else:
                        cpeng.tensor_copy(dst_v, src_v)
                o_sb = sb.tile([16, 8192], f32, tag="o_sb")
                for ch in range(16):
                    op = ps.tile([16, 512], f32, tag="op")
                    nc.tensor.matmul(op, wpoTr,
                                     m_r[:, 512 * ch:512 * (ch + 1)],
                                     start=True, stop=True)
                    eng = getattr(nc, engs[ch % 2])
                    if eng is nc.scalar:
                        eng.copy(o_sb[:, 512 * ch:512 * (ch + 1)], op)
                    else:
                        eng.tensor_copy(o_sb[:, 512 * ch:512 * (ch + 1)], op)
                nc.sync.dma_start(
                    bass.AP(tensor=outq_d.ap().tensor, offset=half * 8192,
                            ap=[[16384, 16], [1, 8192]]),
                    o_sb)

    nc.compile()
    return nc


# --------------------------------------------------------------------------
# host orchestration
# --------------------------------------------------------------------------

def _get(name):
    if name not in _built:
        if name == "p1":
            _built[name] = _build_phase1()
        else:
            _built[name] = _build_phase23(conv_f32r=True)
    return _built[name]


def make_phase1_inputs(x, w_qkv, w_dw, temperature):
    xpads = []
    for b in range(B):
        xpad = np.zeros((16, 258, 258), np.float32)
        xpad[:, 1:257, 1:257] = x[b]
        xpads.append(xpad)
    ins = []
    for k in range(NCORES):
        b, g = divmod(k, 4)
        rows = ([4 * g + u for u in range(4)]
                + [16 + 4 * g + u for u in range(4)]
                + [32 + 4 * g + u for u in range(4)])
        w1big = np.zeros((128, 96), np.float32)
        wdwtap = np.zeros((96, 9), np.float32)
        for bb in range(8):
            for o in range(12):
                w1big[bb * 16:(bb + 1) * 16, o * 8 + bb] = w_qkv[rows[o]]
                wdwtap[o * 8 + bb] = w_dw[rows[o], 0].reshape(9)
        ins.append({
            "xb": xpads[b],
            "w1big": w1big,
            "wdwtap": wdwtap,
            "tempu": np.full((1, 1), temperature[g, 0, 0], np.float32),
        })
    return ins


def _host_eigh(cov_all):
    """cov_all: raw S (32,256,256) f32 -> top-100 eigvecs of
    S/trace(S) + 1e-5 I via jax CPU f64 eigh."""
    import jax
    jax.config.update("jax_enable_x64", True)
    import jax.numpy as jnp
    cpu = jax.devices("cpu")[0]
    s = cov_all.astype(np.float64)
    tra = np.trace(s, axis1=-2, axis2=-1)[:, None, None]
    s = s / tra + 1e-5 * np.eye(256, dtype=np.float64)
    with jax.default_device(cpu):
        _, vecs = jnp.linalg.eigh(jnp.asarray(s))
        U = np.asarray(vecs)[:, :, ::-1][:, :, :TOP_P]
    return U


def _run(name, nc, in_maps):
    from concourse.bass_utils import run_bass_kernel_spmd
    r = run_bass_kernel_spmd(nc, in_maps, core_ids=list(range(NCORES)),
                             trace=PROFILE)
    if PROFILE:
        LAST_PROFILE.append((name, r))
    return r.results


def kernel(x, w_qkv, w_dw, temperature, w_fr, w_po):
    x = np.ascontiguousarray(np.asarray(x, dtype=np.float32))
    w_qkv = np.asarray(w_qkv, dtype=np.float32)
    w_dw = np.asarray(w_dw, dtype=np.float32)
    temperature = np.asarray(temperature, dtype=np.float32)
    w_fr = np.asarray(w_fr, dtype=np.float32)
    w_po = np.asarray(w_po, dtype=np.float32)

    # ---- phase 1
    nc1 = _get("p1")
    in1 = make_phase1_inputs(x, w_qkv, w_dw, temperature)
    res1 = _run("p1", nc1, in1)

    # ---- host eigh
    cov_all = np.zeros((B, C, 256, 256), np.float32)
    vfull = np.zeros((B, C, 256, 256), np.float32)
    for k in range(NCORES):
        b, g = divmod(k, 4)
        cov_all[b, 4 * g:4 * g + 4] = res1[k]["cov"]
        vfull[b, 4 * g:4 * g + 4] = res1[k]["vpd"]
    DEBUG["cov_all"] = cov_all
    DEBUG["vfull"] = vfull
    U = _host_eigh(cov_all.reshape(-1, 256, 256))
    UT = np.ascontiguousarray(
        U.transpose(0, 2, 1).astype(np.float32)).reshape(B, C, TOP_P, 256)

    # ---- phase 2+3
    nc23 = _get("p23")
    wfrT = np.ascontiguousarray(w_fr.T)
    wpoT = np.ascontiguousarray(w_po.T)
    in23 = []
    for k in range(NCORES):
        b, qr = divmod(k, 4)
        in23.append({
            "ut": UT[b],
            "utsl": np.ascontiguousarray(UT[b][:, :, 64 * qr:64 * (qr + 1)]),
            "vall": vfull[b],
            "wfrT": wfrT,
            "wpoT": wpoT,
        })
    try:
        res23 = _run("p23", nc23, in23)
    except Exception:
        # f32r DMA provenance may be rejected by the BIR verifier at
        # NEFF-compile time; fall back to a plain-f32 conv build.
        _built["p23"] = _build_phase23(conv_f32r=False)
        res23 = _run("p23", _built["p23"], in23)

    out = np.zeros((B, C, 256, 256), np.float32)
    for k in range(NCORES):
        b, qr = divmod(k, 4)
        out[b, :, 64 * qr:64 * (qr + 1), :] = res23[k]["outq"]
    return out
